# revision 22
# baseline (speedup 1.0000x reference)
"""nn_GatedDeltaRecurrence Trainium2 kernel (8 NeuronCores, Bass/Tile).

Sharding: core c owns head h=c for both batches (16 (b,h) pairs / 8 cores = 2
pairs per core: data-parallel B x tensor-parallel H per the spec hint). Each
core computes its head's q/k/v/a/b projections + short-conv + norms from the
(host-staged) full inputs, runs the gated delta recurrence in chunked form
(C=128, UT transform, truncated-doubling triangular solve, levels=3), then the
cores exchange per-head outputs with an AllToAll so each core finishes
token-parallel (RMS norm + gate + output projection) for its 512-token slice.
A 16KB AllReduce carries the cross-head sum-of-squares for the RMS norm.

Matmuls run in bf16 with fp32 PSUM accumulation (validated absmax/scale ~5e-3
vs the fp32 reference on this problem's data, well under the 2e-2 gate). The
within-chunk decay exponents L are kept to full fp32 precision on the PE by
splitting them into bf16 hi+lo parts and accumulating rank-1 matmuls.

This build works around a walrus codegen limitation in this container
(instructions with >1 sync-wait commands are rejected) by splitting waits
onto same-engine NOPs at Tile commit time.
"""
import sys

sys.path.insert(0, "/opt/trn_rl_repo")
sys.path.insert(0, "/opt/pypackages")

import numpy as np
import ml_dtypes

B, T = 2, 2048
DM, DKV = 1024, 512
H, KH, VH = 8, 96, 192
KQT, VT = H * KH, H * VH
KS = 4
EPS = 1e-6
C = 128                      # chunk length
NCHUNK = T // C              # 16 chunks per pair
NTOK = B * T                 # 4096 tokens
TOKSLC = NTOK // 8           # 512 tokens per core in the final phase
NEG = -30000.0               # additive mask (exp -> 0)

_CACHE = {}


def _build():
    import bass_rust
    from contextlib import ExitStack
    from concourse import bass, mybir
    from concourse.tile import TileContext
    from concourse.vector_clock import ScopedClock

    F32, BF16 = mybir.dt.float32, mybir.dt.bfloat16
    AL = mybir.AluOpType
    AF = mybir.ActivationFunctionType

    # ---- workarounds: walrus rejects >1 sync-wait per instruction ----
    def _drain_patch(self, tick_clock, wait_clock):
        carrier = self.nc.sync.nop(nofuse=True, hint="drain_waits")
        wait_clock.add_sem_waits(
            carrier.ins, ScopedClock({None: tick_clock.global_clock}))
        si = carrier.ins.sync_info
        waits = list(si.on_wait) if si is not None else []
        if len(waits) > 1:
            carrier.ins.sync_info = bass_rust.SyncInfo(
                on_wait=[waits[0]], on_update=[])
            for w in waits[1:]:
                extra = self.nc.sync.nop(nofuse=True, hint="drain_waits")
                extra.ins.sync_info = bass_rust.SyncInfo(
                    on_wait=[w], on_update=[])
        self.nc.sync.drain()
        self.nc.all_engine_barrier()
        popped = self.nc._tile_sem_poison_stack.pop()
        assert popped is self._sem_poison
        self.nc.clear_and_free_semaphores(
            list(self.sems.allocated().values()))
        self.nc.all_engine_barrier()

    TileContext._drain_and_barrier = _drain_patch
    if not getattr(TileContext, "_split_waits_patched", False):
        _orig_commit = TileContext._commit_instruction

        def _commit_split(self, inst, lazy_reg_writes=True):
            si = getattr(inst, "sync_info", None)
            if (si is not None and si.on_wait
                    and inst.engine != mybir.EngineType.Unassigned
                    and (len(si.on_wait) > 1
                         or isinstance(inst, mybir.InstDrain))):
                waits = list(si.on_wait)
                keep = [] if isinstance(inst, mybir.InstDrain) else [waits.pop(0)]
                for w in waits:
                    nop = mybir.InstNoOp(
                        name=self.nc.get_next_instruction_name(),
                        engine=inst.engine, ins=[], outs=[], debug=inst.debug)
                    nop.sync_info = bass_rust.SyncInfo(on_wait=[w], on_update=[])
                    self.nc.register_instruction(nop, overwrite=True)
                    self._add_instruction(nop)
                inst.sync_info = bass_rust.SyncInfo(
                    on_wait=keep, on_update=list(si.on_update))
            return _orig_commit(self, inst, lazy_reg_writes)

        TileContext._commit_instruction = _commit_split
        TileContext._split_waits_patched = True

    nc = bass.Bass()
    P = {}

    def dp(name, shape, dt):
        P[name] = nc.declare_dram_parameter(name, list(shape), dt, isOutput=False)
        return P[name]

    xT = dp("xT", (DM, NTOK), BF16)
    ckvT = dp("ckvT", (DKV, NTOK), BF16)
    wqab = dp("wqab", (DM, KH + 2), BF16)
    wk = dp("wk", (DKV, KH), BF16)
    wv = dp("wv", (DKV, VH), BF16)
    cqd = dp("cqd", (KS, KH, KH), BF16)
    ckd = dp("ckd", (KS, KH, KH), BF16)
    cvd = dp("cvd", (KS, 2, KH, KH), BF16)
    scal = dp("scal", (1, 8), F32)
    gw = dp("gw", (DM, VT), BF16)
    wo = dp("wo", (VT, DM), BF16)
    xsT = dp("xsT", (DM, TOKSLC), BF16)
    png = dp("png", (VT, 1), F32)
    maskS = dp("maskS", (C, C), F32)
    maskI = dp("maskI", (C, C), F32)
    id128f = dp("id128f", (128, 128), F32)
    id128b = dp("id128b", (128, 128), BF16)
    id96b = dp("id96b", (96, 96), BF16)
    id1b = dp("id1b", (1, 1), BF16)
    onesb = dp("onesb", (1, 128), BF16)
    monesb = dp("monesb", (1, 128), BF16)
    one1b = dp("one1b", (1, 1), BF16)
    out = nc.declare_dram_parameter("out", [TOKSLC, DM], F32, isOutput=True)

    with TileContext(nc, pool_alloc_mode="queue") as tc:
        ctx = ExitStack()
        cst = ctx.enter_context(tc.tile_pool(name="cst", bufs=1))
        pers = ctx.enter_context(tc.tile_pool(name="pers", bufs=1))
        scr = ctx.enter_context(tc.tile_pool(name="scr", bufs=2))
        ring = ctx.enter_context(tc.tile_pool(name="ring", bufs=3))
        strm = ctx.enter_context(tc.tile_pool(name="strm", bufs=3))
        ps_acc = ctx.enter_context(tc.tile_pool(name="ps_acc", bufs=2, space="PSUM"))
        ps_gate = ctx.enter_context(tc.tile_pool(name="ps_gate", bufs=1, space="PSUM"))
        ps_mm = ctx.enter_context(tc.tile_pool(name="ps_mm", bufs=3, space="PSUM"))
        ps_tiny = ctx.enter_context(tc.tile_pool(name="ps_tiny", bufs=2, space="PSUM"))
        dram = ctx.enter_context(tc.tile_pool(name="dram", bufs=1, space="DRAM"))

        def cload(pool, pname, shape, dt, rearr=None):
            t = pool.tile(list(shape), dt, name=pname + "_s")
            src = P[pname][:]
            if rearr is not None:
                src = src.rearrange(rearr[0], **rearr[1])
            nc.sync.dma_start(out=t[:], in_=src)
            return t

        wqab_s = cload(cst, "wqab", (128, 8, KH + 2), BF16,
                       ("(kc p) f -> p kc f", dict(p=128)))
        wk_s = cload(cst, "wk", (128, 4, KH), BF16,
                     ("(kc p) f -> p kc f", dict(p=128)))
        wv_s = cload(cst, "wv", (128, 4, VH), BF16,
                     ("(kc p) f -> p kc f", dict(p=128)))
        cqd_s = cload(cst, "cqd", (KH, KS, KH), BF16, ("s p f -> p s f", {}))
        ckd_s = cload(cst, "ckd", (KH, KS, KH), BF16, ("s p f -> p s f", {}))
        cvd_s = cload(cst, "cvd", (KH, KS, 2, KH), BF16, ("s h p f -> p s h f", {}))
        scal_s = cload(cst, "scal", (1, 8), F32)
        png_s = cload(cst, "png", (128, 12), F32,
                      ("(ct p) o -> p (ct o)", dict(p=128)))
        maskS_s = cload(cst, "maskS", (C, C), F32)
        maskI_s = cload(cst, "maskI", (C, C), F32)
        id128f_s = cload(cst, "id128f", (128, 128), F32)
        id128b_s = cload(cst, "id128b", (128, 128), BF16)
        id96b_s = cload(cst, "id96b", (96, 96), BF16)
        id1b_s = cload(cst, "id1b", (1, 1), BF16)
        onesb_s = cload(cst, "onesb", (1, 128), BF16)
        monesb_s = cload(cst, "monesb", (1, 128), BF16)
        one1b_s = cload(cst, "one1b", (1, 1), BF16)
        ones96b_s = cst.tile([KH, 1], BF16)
        nc.vector.memset(ones96b_s[:], 1.0)
        epsb_s = cst.tile([C, 1], F32)
        nc.vector.memset(epsb_s[:], EPS)
        one1f_s = cst.tile([1, 1], F32)
        nc.vector.memset(one1f_s[:], 1.0)

        # persistent outputs of the scan
        Osb = [pers.tile([C, VH], BF16, name=f"O{j}") for j in range(B * NCHUNK)]
        ssqc = pers.tile([C, B * NCHUNK], F32)
        gate = [pers.tile([128, TOKSLC], BF16, name=f"gate{ct}") for ct in range(12)]

        # ---------------- A) projections ----------------
        work_cm = tc.tile_pool(name="work", bufs=1)
        work = work_cm.__enter__()
        rawp_cm = tc.tile_pool(name="rawp", bufs=1)
        rawp = rawp_cm.__enter__()
        qraw = [rawp.tile([KH, 3 + T], BF16, name=f"qraw{p}") for p in range(B)]
        kraw = [rawp.tile([KH, 3 + T], BF16, name=f"kraw{p}") for p in range(B)]
        v0raw = [rawp.tile([KH, 3 + T], BF16, name=f"v0raw{p}") for p in range(B)]
        v1raw = [rawp.tile([KH, 3 + T], BF16, name=f"v1raw{p}") for p in range(B)]
        abd = [dram.tile([2, T], F32, name=f"abd{p}") for p in range(B)]
        for p in range(B):
            for t_ in (qraw[p], kraw[p], v0raw[p], v1raw[p]):
                nc.vector.memset(t_[:, 0:3], 0.0)

        for tt in range(8):
            p, lt = tt // 4, tt % 4
            ps_q = ps_acc.tile([KH + 2, 512], F32, tag="acc")
            for kc in range(8):
                xt = strm.tile([128, 512], BF16, tag="xt")
                nc.gpsimd.dma_start(
                    out=xt[:], in_=xT[kc * 128:(kc + 1) * 128,
                                      tt * 512:(tt + 1) * 512])
                nc.tensor.matmul(ps_q[:], lhsT=wqab_s[:, kc, :],
                                 rhs=xt[:], start=(kc == 0), stop=(kc == 7))
            nc.scalar.copy(out=qraw[p][:, 3 + lt * 512: 3 + (lt + 1) * 512],
                           in_=ps_q[0:KH, :])
            abev = scr.tile([2, 512], F32, tag="abev", bufs=1)
            nc.scalar.copy(out=abev[:], in_=ps_q[KH:KH + 2, :])
            nc.sync.dma_start(out=abd[p][:, lt * 512:(lt + 1) * 512], in_=abev[:])

            ps_k = ps_acc.tile([KH, 512], F32, tag="acc")
            ps_v0 = ps_mm.tile([KH, 512], F32, tag="mm")
            ps_v1 = ps_mm.tile([KH, 512], F32, tag="mm")
            for kc in range(4):
                ct = strm.tile([128, 512], BF16, tag="ct")
                nc.gpsimd.dma_start(
                    out=ct[:], in_=ckvT[kc * 128:(kc + 1) * 128,
                                        tt * 512:(tt + 1) * 512])
                nc.tensor.matmul(ps_k[:], lhsT=wk_s[:, kc, :],
                                 rhs=ct[:], start=(kc == 0), stop=(kc == 3))
                nc.tensor.matmul(ps_v0[:], lhsT=wv_s[:, kc, 0:KH],
                                 rhs=ct[:], start=(kc == 0), stop=(kc == 3))
                nc.tensor.matmul(ps_v1[:], lhsT=wv_s[:, kc, KH:VH],
                                 rhs=ct[:], start=(kc == 0), stop=(kc == 3))
            nc.scalar.copy(out=kraw[p][:, 3 + lt * 512: 3 + (lt + 1) * 512],
                           in_=ps_k[:])
            nc.scalar.copy(out=v0raw[p][:, 3 + lt * 512: 3 + (lt + 1) * 512],
                           in_=ps_v0[:])
            nc.scalar.copy(out=v1raw[p][:, 3 + lt * 512: 3 + (lt + 1) * 512],
                           in_=ps_v1[:])

        # ---------------- B) conv + SiLU (bf16 outs) ----------------
        qn = [work.tile([KH, T], BF16, name=f"qn{p}") for p in range(B)]
        kn = [work.tile([KH, T], BF16, name=f"kn{p}") for p in range(B)]
        vc0 = [work.tile([KH, T], BF16, name=f"vc0{p}") for p in range(B)]
        vc1 = [work.tile([KH, T], BF16, name=f"vc1{p}") for p in range(B)]
        conv_jobs = []
        for p in range(B):
            conv_jobs += [(qraw[p], cqd_s, qn[p], None),
                          (kraw[p], ckd_s, kn[p], None),
                          (v0raw[p], cvd_s, vc0[p], 0),
                          (v1raw[p], cvd_s, vc1[p], 1)]
        for raw, dg, dst, vh in conv_jobs:
            for lt in range(4):
                ps_c = ps_mm.tile([KH, 512], F32, tag="mm")
                for s in range(KS):
                    lhs = dg[:, s, vh, :] if vh is not None else dg[:, s, :]
                    nc.tensor.matmul(ps_c[:], lhsT=lhs,
                                     rhs=raw[:, lt * 512 + s: lt * 512 + s + 512],
                                     start=(s == 0), stop=(s == KS - 1))
                nc.scalar.activation(out=dst[:, lt * 512:(lt + 1) * 512],
                                     in_=ps_c[:], func=AF.Silu)
        rawp_cm.__exit__(None, None, None)

        # gate weights reuse the raw tiles' space (gate GEMM overlaps the scan)
        mid_cm = tc.tile_pool(name="mid", bufs=1)
        mid = mid_cm.__enter__()
        gw_s = [mid.tile([128, VT], BF16, name=f"gwt{kc}") for kc in range(8)]
        xs_s = [mid.tile([128, TOKSLC], BF16, name=f"xst{kc}") for kc in range(8)]
        for kcb in range(8):
            nc.gpsimd.dma_start(out=gw_s[kcb][:],
                                in_=gw[kcb * 128:(kcb + 1) * 128, :])
            nc.gpsimd.dma_start(out=xs_s[kcb][:],
                                in_=xsT[kcb * 128:(kcb + 1) * 128, :])

        # ---------------- C) L2 norm of q,k (in place) ----------------
        for p in range(B):
            for src, qscale in ((qn[p], KH ** -0.5), (kn[p], None)):
                for lt in range(4):
                    sl = slice(lt * 512, (lt + 1) * 512)
                    sq = scr.tile([KH, 512], BF16, tag="sq")
                    nc.vector.tensor_tensor(out=sq[:], in0=src[:, sl],
                                            in1=src[:, sl], op=AL.mult)
                    ps_n = ps_tiny.tile([1, 512], F32, tag="tiny")
                    nc.tensor.matmul(ps_n[:], lhsT=ones96b_s[:], rhs=sq[:],
                                     start=True, stop=True)
                    # factor = qscale/sqrt(ssq) = rsqrt(ssq/qscale^2): one ACT
                    # Rsqrt (eps negligible: ||q|| >> 1e-6 for silu-conv outs).
                    nrb = scr.tile([1, 512], BF16, tag="recb")
                    iscale = (1.0 / float(qscale) ** 2 if qscale is not None
                              else 1.0)
                    eng = nc.scalar
                    eng.add_instruction(mybir.InstActivation(
                        name=nc.get_next_instruction_name(),
                        func=AF.Rsqrt,
                        ins=[eng.lower_ap(ps_n[:]),
                             mybir.ImmediateValue(dtype=F32, value=0.0),
                             mybir.ImmediateValue(dtype=F32, value=iscale),
                             mybir.ImmediateValue(dtype=F32, value=0.0)],
                        outs=[eng.lower_ap(nrb[:])]))
                    ps_f = ps_mm.tile([KH, 512], F32, tag="mm")
                    nc.tensor.matmul(ps_f[:], lhsT=onesb_s[0:1, 0:KH],
                                     rhs=nrb[:], start=True, stop=True)
                    nc.vector.tensor_tensor(out=src[:, sl], in0=src[:, sl],
                                            in1=ps_f[:], op=AL.mult)

        # ---------------- D) decay rows + cumsum (bf16 hi/lo rows) -----------
        Lh = [work.tile([1, T], BF16, name=f"Lh{p}") for p in range(B)]
        Ll = [work.tile([1, T], BF16, name=f"Ll{p}") for p in range(B)]
        brow = [work.tile([1, T], BF16, name=f"brow{p}") for p in range(B)]
        lnbrow = [work.tile([1, T], BF16, name=f"lnbrow{p}") for p in range(B)]
        for p in range(B):
            arow = scr.tile([1, T], F32, tag="rowC", bufs=1)
            nc.sync.dma_start(out=arow[:], in_=abd[p][0:1, :])
            rsc0 = scr.tile([1, T], F32, tag="rowD", bufs=1)
            nc.scalar.activation(out=rsc0[:], in_=arow[:], func=AF.Exp,
                                 bias=scal_s[0:1, 0:1], scale=1.0)
            rsc = scr.tile([1, T], F32, tag="rowC", bufs=1)
            nc.scalar.activation(out=rsc[:], in_=rsc0[:], func=AF.Ln,
                                 bias=one1f_s[:], scale=1.0)
            brawrow = scr.tile([1, T], F32, tag="rowC", bufs=1)
            nc.sync.dma_start(out=brawrow[:], in_=abd[p][1:2, :])
            grow = scr.tile([1, T], F32, tag="rowB", bufs=1)
            nc.vector.tensor_scalar(out=grow[:], in0=rsc[:],
                                    scalar1=scal_s[0:1, 1:2], scalar2=None,
                                    op0=AL.mult)
            nc.scalar.activation(out=brow[p][:], in_=brawrow[:],
                                 func=AF.Sigmoid, bias=scal_s[0:1, 2:3], scale=1.0)
            lnb_e = scr.tile([1, T], F32, tag="rowD", bufs=1)
            nc.scalar.activation(out=lnb_e[:], in_=brawrow[:], func=AF.Exp,
                                 bias=scal_s[0:1, 3:4], scale=-1.0)
            lnb_t = scr.tile([1, T], F32, tag="rowC", bufs=1)
            nc.scalar.activation(out=lnb_t[:], in_=lnb_e[:], func=AF.Ln,
                                 bias=one1f_s[:], scale=1.0)
            nc.vector.tensor_scalar_mul(out=lnbrow[p][:], in0=lnb_t[:], scalar1=-1.0)
            gb = dram.tile([NCHUNK, C], F32, name=f"gb{p}")
            nc.sync.dma_start(
                out=gb[:].rearrange("p f -> (p f)").unsqueeze(0), in_=grow[:])
            g16 = scr.tile([NCHUNK, C], F32, tag="g16")
            nc.sync.dma_start(out=g16[:], in_=gb[:])
            L16t = scr.tile([NCHUNK, C], F32, tag="L16t")
            nc.vector.tensor_tensor_scan(out=L16t[:], data0=g16[:], data1=g16[:],
                                         initial=0.0, op0=AL.add, op1=AL.bypass)
            gb2 = dram.tile([NCHUNK, C], F32, name=f"gb2{p}")
            nc.sync.dma_start(out=gb2[:], in_=L16t[:])
            Lrow = scr.tile([1, T], F32, tag="rowC", bufs=1)
            nc.sync.dma_start(
                out=Lrow[:], in_=gb2[:].rearrange("p f -> (p f)").unsqueeze(0))
            nc.vector.tensor_copy(out=Lh[p][:], in_=Lrow[:])
            nc.vector.tensor_tensor(out=Ll[p][:], in0=Lrow[:], in1=Lh[p][:],
                                    op=AL.subtract)

        # -------- E/F/G) per-chunk prep + sequential sweep + output ----------
        Scur = []
        for p in range(B):
            s0 = ring.tile([KH, VH], BF16, tag=f"Sh{p}", bufs=4, name=f"S0_{p}")
            nc.vector.memset(s0[:], 0.0)
            Scur.append(s0)

        for i in range(NCHUNK):
            for p in range(B):
                j = i * B + p
                ck = slice(i * C, (i + 1) * C)
                lstc = slice((i + 1) * C - 1, (i + 1) * C)
                Lrh, Lrl = Lh[p][0:1, ck], Ll[p][0:1, ck]

                # E[t,s] = L_t - L_s via 4 rank-1 bf16 matmuls (hi/lo exact)
                ps_e = ps_mm.tile([C, C], F32, tag="mm")
                nc.tensor.matmul(ps_e[:], lhsT=Lrh, rhs=onesb_s[:],
                                 start=True, stop=False)
                nc.tensor.matmul(ps_e[:], lhsT=Lrl, rhs=onesb_s[:],
                                 start=False, stop=False)
                nc.tensor.matmul(ps_e[:], lhsT=monesb_s[:], rhs=Lrh,
                                 start=False, stop=False)
                nc.tensor.matmul(ps_e[:], lhsT=monesb_s[:], rhs=Lrl,
                                 start=False, stop=True)
                Es = scr.tile([C, C], F32, tag="Es", bufs=3)
                nc.vector.tensor_tensor(out=Es[:], in0=ps_e[:], in1=maskS_s[:],
                                        op=AL.add)
                Ei = scr.tile([C, C], F32, tag="Ei", bufs=3)
                nc.vector.tensor_tensor(out=Ei[:], in0=ps_e[:], in1=maskI_s[:],
                                        op=AL.add)

                # lnb / beta columns
                ps_lc = ps_tiny.tile([C, 1], BF16, tag="tiny")
                nc.tensor.transpose(ps_lc[:], lnbrow[p][0:1, ck], id1b_s[:])
                lnbc = scr.tile([C, 1], F32, tag="lnbc")
                nc.scalar.copy(out=lnbc[:], in_=ps_lc[:])
                ps_bc = ps_tiny.tile([C, 1], BF16, tag="tiny")
                nc.tensor.transpose(ps_bc[:], brow[p][0:1, ck], id1b_s[:])
                bc = scr.tile([C, 1], F32, tag="bc")
                nc.scalar.copy(out=bc[:], in_=ps_bc[:])

                Mexp = scr.tile([C, C], F32, tag="Mexp", bufs=3)
                nc.scalar.activation(out=Mexp[:], in_=Es[:], func=AF.Exp,
                                     bias=lnbc[:], scale=1.0)
                Dincl = scr.tile([C, C], F32, tag="Dincl", bufs=3)
                nc.scalar.activation(out=Dincl[:], in_=Ei[:], func=AF.Exp)

                # N (strict-lower, includes beta) and its transpose
                ps_gk = ps_mm.tile([C, C], F32, tag="mm")
                nc.tensor.matmul(ps_gk[:], lhsT=kn[p][:, ck], rhs=kn[p][:, ck],
                                 start=True, stop=True)
                Nbf = scr.tile([C, C], BF16, tag="Nbf", bufs=3)
                nc.vector.scalar_tensor_tensor(out=Nbf[:], in0=ps_gk[:],
                                               scalar=-1.0, in1=Mexp[:],
                                               op0=AL.mult, op1=AL.mult)
                # P = (q_t . k_s) * Dincl
                ps_gq = ps_mm.tile([C, C], F32, tag="mm")
                nc.tensor.matmul(ps_gq[:], lhsT=qn[p][:, ck], rhs=kn[p][:, ck],
                                 start=True, stop=True)
                Pbf = scr.tile([C, C], BF16, tag="Pbf", bufs=3)
                nc.vector.tensor_tensor(out=Pbf[:], in0=ps_gq[:], in1=Dincl[:],
                                        op=AL.mult)
                # both transposes (N, P) share one psum bank
                ps_nt = ps_mm.tile([C, 2, C], BF16, tag="mm")
                nc.tensor.transpose(ps_nt[:, 0], Nbf[:], id128b_s[:])
                nc.tensor.transpose(ps_nt[:, 1], Pbf[:], id128b_s[:])
                ev = nc.scalar.copy if (j % 2 == 0) else \
                    (lambda out, in_: nc.vector.tensor_copy(out=out, in_=in_))
                NTbf = scr.tile([C, C], BF16, tag="NTbf", bufs=3)
                ev(out=NTbf[:], in_=ps_nt[:, 0])
                P0b = scr.tile([C, C], BF16, tag="P0b", bufs=3)
                nc.vector.tensor_tensor(out=P0b[:], in0=ps_nt[:, 0],
                                        in1=id128f_s[:], op=AL.add)
                PTb = ring.tile([C, C], BF16, tag="PTb", name=f"PTb{j}")
                nc.scalar.copy(out=PTb[:], in_=ps_nt[:, 1])

                # doubling powers (levels=3)
                ps_sq = ps_mm.tile([C, C], F32, tag="mm")
                nc.tensor.matmul(ps_sq[:], lhsT=NTbf[:], rhs=Nbf[:],
                                 start=True, stop=True)
                N2r = scr.tile([C, C], BF16, tag="N2r", bufs=3)
                ev(out=N2r[:], in_=ps_sq[:])
                N2i = scr.tile([C, C], BF16, tag="N2i", bufs=3)
                nc.vector.tensor_tensor(out=N2i[:], in0=ps_sq[:], in1=id128f_s[:],
                                        op=AL.add)
                ps_sq2 = ps_mm.tile([C, C], F32, tag="mm")
                nc.tensor.matmul(ps_sq2[:], lhsT=Nbf[:], rhs=NTbf[:],
                                 start=True, stop=True)
                N2Tr = scr.tile([C, C], BF16, tag="N2Tr", bufs=3)
                ev(out=N2Tr[:], in_=ps_sq2[:])
                ps_sq3 = ps_mm.tile([C, C], F32, tag="mm")
                nc.tensor.matmul(ps_sq3[:], lhsT=N2Tr[:], rhs=N2r[:],
                                 start=True, stop=True)
                N4r = scr.tile([C, C], BF16, tag="N4r", bufs=3)
                ev(out=N4r[:], in_=ps_sq3[:])
                N4i = scr.tile([C, C], BF16, tag="N4i", bufs=3)
                nc.vector.tensor_tensor(out=N4i[:], in0=ps_sq3[:], in1=id128f_s[:],
                                        op=AL.add)
                ps_sq4 = ps_mm.tile([C, C], F32, tag="mm")
                nc.tensor.matmul(ps_sq4[:], lhsT=N2r[:], rhs=N2Tr[:],
                                 start=True, stop=True)
                N4Tr = scr.tile([C, C], BF16, tag="N4Tr", bufs=3)
                ev(out=N4Tr[:], in_=ps_sq4[:])
                ps_sq5 = ps_mm.tile([C, C], F32, tag="mm")
                nc.tensor.matmul(ps_sq5[:], lhsT=N4Tr[:], rhs=N4r[:],
                                 start=True, stop=True)
                N8i = scr.tile([C, C], BF16, tag="N8i", bufs=3)
                nc.vector.tensor_tensor(out=N8i[:], in0=ps_sq5[:], in1=id128f_s[:],
                                        op=AL.add)

                # chain: T^T = (I+N8T)(I+N4T)(I+N2T)(I+NT)
                ps_c1 = ps_mm.tile([C, C], F32, tag="mm")
                nc.tensor.matmul(ps_c1[:], lhsT=N2i[:], rhs=P0b[:],
                                 start=True, stop=True)
                C1 = scr.tile([C, C], BF16, tag="C1", bufs=3)
                ev(out=C1[:], in_=ps_c1[:])
                ps_c2 = ps_mm.tile([C, C], F32, tag="mm")
                nc.tensor.matmul(ps_c2[:], lhsT=N4i[:], rhs=C1[:],
                                 start=True, stop=True)
                C2 = scr.tile([C, C], BF16, tag="C2", bufs=3)
                ev(out=C2[:], in_=ps_c2[:])
                ps_c3 = ps_mm.tile([C, C], F32, tag="mm")
                nc.tensor.matmul(ps_c3[:], lhsT=N8i[:], rhs=C2[:],
                                 start=True, stop=True)
                TTm = ring.tile([C, C], BF16, tag="TTm", name=f"TTm{j}")
                nc.scalar.copy(out=TTm[:], in_=ps_c3[:])

                # V halves + K token-layout transposes share one psum bank
                bV = ring.tile([C, VH], BF16, tag="bV", name=f"bV{j}")
                ps_vt = ps_mm.tile([C, 3, KH], BF16, tag="mm")
                nc.tensor.transpose(ps_vt[:, 0], vc0[p][:, ck], id96b_s[:])
                nc.tensor.transpose(ps_vt[:, 1], vc1[p][:, ck], id96b_s[:])
                nc.tensor.transpose(ps_vt[:, 2], kn[p][:, ck], id96b_s[:])
                for hh in range(2):
                    nc.vector.tensor_scalar(out=bV[:, hh * KH:(hh + 1) * KH],
                                            in0=ps_vt[:, hh], scalar1=bc[:],
                                            scalar2=None, op0=AL.mult)
                ps_kt = ps_vt[:, 2]
                ps_ll = ps_tiny.tile([C, 1], F32, tag="tiny")
                nc.tensor.matmul(ps_ll[:], lhsT=onesb_s[:], rhs=Lh[p][0:1, lstc],
                                 start=True, stop=False)
                nc.tensor.matmul(ps_ll[:], lhsT=onesb_s[:], rhs=Ll[p][0:1, lstc],
                                 start=False, stop=True)
                Llc = scr.tile([C, 1], F32, tag="Llc")
                nc.scalar.copy(out=Llc[:], in_=ps_ll[:])
                ps_lcol = ps_tiny.tile([C, 1], F32, tag="tiny")
                nc.tensor.matmul(ps_lcol[:], lhsT=Lrh, rhs=one1b_s[:],
                                 start=True, stop=False)
                nc.tensor.matmul(ps_lcol[:], lhsT=Lrl, rhs=one1b_s[:],
                                 start=False, stop=True)
                eLl = scr.tile([C, 1], F32, tag="eLl")
                nc.scalar.activation(out=eLl[:], in_=ps_lcol[:], func=AF.Exp,
                                     bias=Llc[:], scale=-1.0)
                Ktok = ring.tile([C, KH], BF16, tag="Ktok", name=f"Ktok{j}")
                nc.vector.tensor_scalar(out=Ktok[:], in0=ps_kt, scalar1=eLl[:],
                                        scalar2=None, op0=AL.mult)

                # A_t column, beta*A column, chunk-total decay
                Acol = ring.tile([C, 1], F32, tag="Acol", name=f"Acol{j}")
                nc.scalar.activation(out=Acol[:], in_=ps_lcol[:], func=AF.Exp)
                bAcol = ring.tile([C, 1], F32, tag="bAcol", name=f"bAcol{j}")
                nc.vector.tensor_tensor(out=bAcol[:], in0=Acol[:], in1=bc[:],
                                        op=AL.mult)
                ps_ac = ps_tiny.tile([KH, 1], F32, tag="tiny")
                nc.tensor.matmul(ps_ac[:], lhsT=onesb_s[0:1, 0:KH],
                                 rhs=Lh[p][0:1, lstc], start=True, stop=False)
                nc.tensor.matmul(ps_ac[:], lhsT=onesb_s[0:1, 0:KH],
                                 rhs=Ll[p][0:1, lstc], start=False, stop=True)
                aC96 = ring.tile([KH, 1], F32, tag="aC96", name=f"aC96{j}")
                nc.scalar.activation(out=aC96[:], in_=ps_ac[:], func=AF.Exp)

                # ---- sequential sweep step ----
                ps_y = ps_mm.tile([C, VH], F32, tag="mm")
                nc.tensor.matmul(ps_y[:], lhsT=kn[p][:, ck], rhs=Scur[p][:],
                                 start=True, stop=True)
                R2 = scr.tile([C, VH], BF16, tag="R2")
                nc.vector.scalar_tensor_tensor(out=R2[:], in0=ps_y[:],
                                               scalar=bAcol[:], in1=bV[:],
                                               op0=AL.mult, op1=AL.subtract)
                ps_u = ps_mm.tile([C, VH], F32, tag="mm")
                nc.tensor.matmul(ps_u[:], lhsT=TTm[:], rhs=R2[:],
                                 start=True, stop=True)
                U = ring.tile([C, VH], BF16, tag="U", name=f"U{j}")
                nc.scalar.activation(out=U[:], in_=ps_u[:], func=AF.Copy,
                                     scale=-1.0)
                ps_s = ps_mm.tile([KH, VH], F32, tag="mm")
                nc.tensor.matmul(ps_s[:], lhsT=Ktok[:], rhs=U[:],
                                 start=True, stop=True)
                Snew = ring.tile([KH, VH], BF16, tag=f"Sh{p}", bufs=4,
                                 name=f"S{p}_{i + 1}")
                nc.vector.scalar_tensor_tensor(out=Snew[:], in0=Scur[p][:],
                                               scalar=aC96[:], in1=ps_s[:],
                                               op0=AL.mult, op1=AL.add)

                # ---- output epilogue ----
                ps_pu = ps_acc.tile([C, VH], F32, tag="acc")
                nc.tensor.matmul(ps_pu[:], lhsT=PTb[:], rhs=U[:],
                                 start=True, stop=True)
                ps_z = ps_acc.tile([C, VH], F32, tag="acc")
                nc.tensor.matmul(ps_z[:], lhsT=qn[p][:, ck], rhs=Scur[p][:],
                                 start=True, stop=True)
                tmpZ = scr.tile([C, VH], F32, tag="tmpZ")
                nc.vector.tensor_scalar(out=tmpZ[:], in0=ps_z[:], scalar1=Acol[:],
                                        scalar2=None, op0=AL.mult)
                nc.vector.tensor_tensor(out=Osb[j][:], in0=ps_pu[:], in1=tmpZ[:],
                                        op=AL.add)
                sqo = scr.tile([C, VH], BF16, tag="sqo")
                nc.vector.scalar_tensor_tensor(out=sqo[:], in0=Osb[j][:],
                                               scalar=1.0, in1=Osb[j][:],
                                               op0=AL.mult, op1=AL.mult,
                                               accum_out=ssqc[:, j:j + 1])
                Scur[p] = Snew

        # ---------------- K) gate GEMM (overlaps scan via dataflow) ----------
        for ctb in range(12):
            ps_g = ps_gate.tile([128, TOKSLC], F32, tag="gate")
            for kcb in range(8):
                nc.tensor.matmul(ps_g[:],
                                 lhsT=gw_s[kcb][:, ctb * 128:(ctb + 1) * 128],
                                 rhs=xs_s[kcb][:], start=(kcb == 0), stop=(kcb == 7))
            nc.scalar.activation(out=gate[ctb][:], in_=ps_g[:], func=AF.Silu)

        # ---------------- H) ssq AllReduce + rsqrt ----------------
        arin = dram.tile([C, B * NCHUNK], F32, name="arin")
        arout = dram.tile([C, B * NCHUNK], F32, name="arout")
        nc.sync.dma_start(out=arin[:], in_=ssqc[:])
        nc.gpsimd.collective_compute(
            "AllReduce", AL.add, replica_groups=[list(range(8))],
            ins=[arin.opt()], outs=[arout.opt()])
        rq = pers.tile([C, B * NCHUNK], F32)
        nc.sync.dma_start(out=rq[:], in_=arout[:])
        rb = pers.tile([C, B * NCHUNK], F32)
        nc.scalar.activation(out=rb[:], in_=rq[:], func=AF.Sqrt,
                             bias=epsb_s[:], scale=1.0 / VT)
        rs = pers.tile([C, B * NCHUNK], F32)
        nc.vector.reciprocal(out=rs[:], in_=rb[:])

        # ---------------- I) normalize + transpose + A2A ----------------
        a2ain = dram.tile([8, VH, TOKSLC], BF16, name="a2ain")
        a2aout = dram.tile([8, VH, TOKSLC], BF16, name="a2aout")
        for j in range(B * NCHUNK):
            p, i = j % B, j // B
            jj = i * B + p
            tok0 = p * T + i * C
            d, off = tok0 // TOKSLC, tok0 % TOKSLC
            On = scr.tile([C, VH], BF16, tag="On")
            nc.vector.tensor_scalar(out=On[:], in0=Osb[jj][:],
                                    scalar1=rs[:, jj:jj + 1],
                                    scalar2=None, op0=AL.mult)
            for hh in range(2):
                ps_ot = ps_mm.tile([KH, C], BF16, tag="mm")
                nc.tensor.transpose(ps_ot[:], On[:, hh * KH:(hh + 1) * KH],
                                    id128b_s[:])
                otb = scr.tile([KH, C], BF16, tag="otb")
                nc.scalar.copy(out=otb[:], in_=ps_ot[:])
                nc.sync.dma_start(
                    out=a2ain[d, hh * KH:(hh + 1) * KH, off:off + C], in_=otb[:])
        nc.gpsimd.collective_compute(
            "AllToAll", AL.bypass, replica_groups=[list(range(8))],
            ins=[a2ain.opt()], outs=[a2aout.opt()])

        mid_cm.__exit__(None, None, None)
        work_cm.__exit__(None, None, None)

        # ---------------- L) OG product + final GEMM ----------------
        late_cm = tc.tile_pool(name="late", bufs=1)
        late = late_cm.__enter__()
        wo_s = [late.tile([128, DM], BF16, name=f"wot{ct}") for ct in range(12)]
        for ct in range(12):
            nc.gpsimd.dma_start(out=wo_s[ct][:],
                                in_=wo[ct * 128:(ct + 1) * 128, :])
        og = [late.tile([128, TOKSLC], BF16, name=f"og{ct}") for ct in range(12)]
        a2a_flat = a2aout[:].rearrange("h c t -> (h c) t")
        for ct in range(12):
            ogin = late.tile([128, TOKSLC], BF16, tag="ogin", bufs=2, name=f"ogin{ct}")
            nc.gpsimd.dma_start(out=ogin[:],
                                in_=a2a_flat[ct * 128:(ct + 1) * 128, :])
            nc.vector.scalar_tensor_tensor(
                out=og[ct][:], in0=ogin[:], scalar=png_s[:, ct:ct + 1],
                in1=gate[ct][:], op0=AL.mult, op1=AL.mult)
        for to in range(4):
            for fo in range(2):
                ps_o = ps_gate.tile([128, 512], F32, tag="gate")
                for ct in range(12):
                    nc.tensor.matmul(ps_o[:],
                                     lhsT=og[ct][:, to * 128:(to + 1) * 128],
                                     rhs=wo_s[ct][:, fo * 512:(fo + 1) * 512],
                                     start=(ct == 0), stop=(ct == 11))
                osb = late.tile([128, 512], F32, tag="osb", bufs=2, name=f"osb{to}_{fo}")
                nc.scalar.copy(out=osb[:], in_=ps_o[:])
                nc.sync.dma_start(
                    out=out[to * 128:(to + 1) * 128, fo * 512:(fo + 1) * 512],
                    in_=osb[:])
        late_cm.__exit__(None, None, None)
        ctx.close()

    return nc


def kernel(x, c_kv, w_q, w_k, w_v, conv_q_w, conv_q_b, conv_k_w, conv_k_b,
           conv_v_w, conv_v_b, a_proj_w, a_proj_b, A_log, dt_bias,
           b_proj_w, b_proj_b, g_proj_w, post_norm_w, w_o):
    from concourse.bass_utils import run_bass_kernel_spmd

    bf = ml_dtypes.bfloat16
    x = np.asarray(x, np.float32)
    c_kv = np.asarray(c_kv, np.float32)
    xT = np.ascontiguousarray(x.reshape(NTOK, DM).T).astype(bf)
    ckvT = np.ascontiguousarray(c_kv.reshape(NTOK, DKV).T).astype(bf)
    gw = np.asarray(g_proj_w, np.float32).astype(bf)
    wo_ = np.asarray(w_o, np.float32).astype(bf)
    png = np.asarray(post_norm_w, np.float32).reshape(VT, 1)

    maskS = np.where(np.arange(C)[None, :] < np.arange(C)[:, None], 0.0, NEG)
    maskI = np.where(np.arange(C)[None, :] <= np.arange(C)[:, None], 0.0, NEG)
    consts = dict(
        maskS=maskS.astype(np.float32), maskI=maskI.astype(np.float32),
        id128f=np.eye(128, dtype=np.float32),
        id128b=np.eye(128, dtype=np.float32).astype(bf),
        id96b=np.eye(96, dtype=np.float32).astype(bf),
        id1b=np.ones((1, 1), np.float32).astype(bf),
        onesb=np.ones((1, 128), np.float32).astype(bf),
        monesb=(-np.ones((1, 128), np.float32)).astype(bf),
        one1b=np.ones((1, 1), np.float32).astype(bf),
    )

    in_maps = []
    for c in range(8):
        h = c
        qs = slice(h * KH, (h + 1) * KH)
        vs = slice(h * VH, (h + 1) * VH)
        wqab_ = np.concatenate([
            np.asarray(w_q, np.float32)[:, qs],
            np.asarray(a_proj_w, np.float32)[:, h:h + 1],
            np.asarray(b_proj_w, np.float32)[:, h:h + 1]], axis=1).astype(bf)
        cq = np.asarray(conv_q_w, np.float32)[qs, 0, :]
        ck = np.asarray(conv_k_w, np.float32)[qs, 0, :]
        cv = np.asarray(conv_v_w, np.float32)[vs, 0, :]
        cqd_ = np.stack([np.diag(cq[:, s]) for s in range(KS)]).astype(bf)
        ckd_ = np.stack([np.diag(ck[:, s]) for s in range(KS)]).astype(bf)
        cvd_ = np.stack([np.stack([np.diag(cv[hh * KH:(hh + 1) * KH, s])
                                   for hh in range(2)])
                         for s in range(KS)]).astype(bf)
        scal_ = np.zeros((1, 8), np.float32)
        scal_[0, 0] = float(np.asarray(dt_bias)[h] + np.asarray(a_proj_b)[h])
        scal_[0, 1] = -float(np.exp(np.asarray(A_log)[h]))
        scal_[0, 2] = float(np.asarray(b_proj_b)[h])
        scal_[0, 3] = -float(np.asarray(b_proj_b)[h])
        m = dict(
            xT=xT, ckvT=ckvT, wqab=wqab_,
            wk=np.asarray(w_k, np.float32)[:, qs].astype(bf),
            wv=np.asarray(w_v, np.float32)[:, vs].astype(bf),
            cqd=cqd_, ckd=ckd_, cvd=cvd_, scal=scal_, gw=gw, wo=wo_,
            xsT=np.ascontiguousarray(xT[:, c * TOKSLC:(c + 1) * TOKSLC]),
            png=png, **consts)
        in_maps.append(m)

    if "nc" not in _CACHE:
        _CACHE["nc"] = _build()
    res = run_bass_kernel_spmd(_CACHE["nc"], in_maps, core_ids=list(range(8)))
    _CACHE["last"] = res
    parts = [np.asarray(res.results[c]["out"], np.float32) for c in range(8)]
    return np.concatenate(parts, axis=0).reshape(B, T, DM)


# revision 26
# speedup vs baseline: 1.0162x; 1.0162x over previous
"""nn_GatedDeltaRecurrence Trainium2 kernel (8 NeuronCores, Bass/Tile).

Sharding: core c owns head h=c for both batches (16 (b,h) pairs / 8 cores = 2
pairs per core: data-parallel B x tensor-parallel H per the spec hint). Each
core computes its head's q/k/v/a/b projections + short-conv + norms from the
(host-staged) full inputs, runs the gated delta recurrence in chunked form
(C=128, UT transform, truncated-doubling triangular solve, levels=3), then the
cores exchange per-head outputs with an AllToAll so each core finishes
token-parallel (RMS norm + gate + output projection) for its 512-token slice.
A 16KB AllReduce carries the cross-head sum-of-squares for the RMS norm.

Matmuls run in bf16 with fp32 PSUM accumulation (validated absmax/scale ~5e-3
vs the fp32 reference on this problem's data, well under the 2e-2 gate). The
within-chunk decay exponents L are kept to full fp32 precision on the PE by
splitting them into bf16 hi+lo parts and accumulating rank-1 matmuls.

This build works around a walrus codegen limitation in this container
(instructions with >1 sync-wait commands are rejected) by splitting waits
onto same-engine NOPs at Tile commit time.
"""
import sys

sys.path.insert(0, "/opt/trn_rl_repo")
sys.path.insert(0, "/opt/pypackages")

import numpy as np
import ml_dtypes

B, T = 2, 2048
DM, DKV = 1024, 512
H, KH, VH = 8, 96, 192
KQT, VT = H * KH, H * VH
KS = 4
EPS = 1e-6
C = 128                      # chunk length
NCHUNK = T // C              # 16 chunks per pair
NTOK = B * T                 # 4096 tokens
TOKSLC = NTOK // 8           # 512 tokens per core in the final phase
NEG = -30000.0               # additive mask (exp -> 0)

_CACHE = {}


def _build():
    import bass_rust
    from contextlib import ExitStack
    from concourse import bass, mybir
    from concourse.tile import TileContext
    from concourse.vector_clock import ScopedClock

    F32, BF16 = mybir.dt.float32, mybir.dt.bfloat16
    AL = mybir.AluOpType
    AF = mybir.ActivationFunctionType

    # ---- workarounds: walrus rejects >1 sync-wait per instruction ----
    def _drain_patch(self, tick_clock, wait_clock):
        carrier = self.nc.sync.nop(nofuse=True, hint="drain_waits")
        wait_clock.add_sem_waits(
            carrier.ins, ScopedClock({None: tick_clock.global_clock}))
        si = carrier.ins.sync_info
        waits = list(si.on_wait) if si is not None else []
        if len(waits) > 1:
            carrier.ins.sync_info = bass_rust.SyncInfo(
                on_wait=[waits[0]], on_update=[])
            for w in waits[1:]:
                extra = self.nc.sync.nop(nofuse=True, hint="drain_waits")
                extra.ins.sync_info = bass_rust.SyncInfo(
                    on_wait=[w], on_update=[])
        self.nc.sync.drain()
        self.nc.all_engine_barrier()
        popped = self.nc._tile_sem_poison_stack.pop()
        assert popped is self._sem_poison
        self.nc.clear_and_free_semaphores(
            list(self.sems.allocated().values()))
        self.nc.all_engine_barrier()

    TileContext._drain_and_barrier = _drain_patch
    if not getattr(TileContext, "_split_waits_patched", False):
        _orig_commit = TileContext._commit_instruction

        def _commit_split(self, inst, lazy_reg_writes=True):
            si = getattr(inst, "sync_info", None)
            if (si is not None and si.on_wait
                    and inst.engine != mybir.EngineType.Unassigned
                    and (len(si.on_wait) > 1
                         or isinstance(inst, mybir.InstDrain))):
                waits = list(si.on_wait)
                keep = [] if isinstance(inst, mybir.InstDrain) else [waits.pop(0)]
                for w in waits:
                    nop = mybir.InstNoOp(
                        name=self.nc.get_next_instruction_name(),
                        engine=inst.engine, ins=[], outs=[], debug=inst.debug)
                    nop.sync_info = bass_rust.SyncInfo(on_wait=[w], on_update=[])
                    self.nc.register_instruction(nop, overwrite=True)
                    self._add_instruction(nop)
                inst.sync_info = bass_rust.SyncInfo(
                    on_wait=keep, on_update=list(si.on_update))
            return _orig_commit(self, inst, lazy_reg_writes)

        TileContext._commit_instruction = _commit_split
        TileContext._split_waits_patched = True

    nc = bass.Bass()
    P = {}

    def dp(name, shape, dt):
        P[name] = nc.declare_dram_parameter(name, list(shape), dt, isOutput=False)
        return P[name]

    xT = dp("xT", (DM, NTOK), BF16)
    ckvT = dp("ckvT", (DKV, NTOK), BF16)
    wqab = dp("wqab", (DM, KH + 2), BF16)
    wk = dp("wk", (DKV, KH), BF16)
    wv = dp("wv", (DKV, VH), BF16)
    cqd = dp("cqd", (KS, KH, KH), BF16)
    ckd = dp("ckd", (KS, KH, KH), BF16)
    cvd = dp("cvd", (KS, 2, KH, KH), BF16)
    scal = dp("scal", (1, 8), F32)
    gw = dp("gw", (DM, VT), BF16)
    wo = dp("wo", (VT, DM), BF16)
    xsT = dp("xsT", (DM, TOKSLC), BF16)
    png = dp("png", (VT, 1), F32)
    maskS = dp("maskS", (C, C), F32)
    maskI = dp("maskI", (C, C), F32)
    id128f = dp("id128f", (128, 128), F32)
    id128b = dp("id128b", (128, 128), BF16)
    id96b = dp("id96b", (96, 96), BF16)
    id1b = dp("id1b", (1, 1), BF16)
    onesb = dp("onesb", (1, 128), BF16)
    monesb = dp("monesb", (1, 128), BF16)
    one1b = dp("one1b", (1, 1), BF16)
    selq = dp("selq", (B * NCHUNK, 4), F32)
    id4f = dp("id4f", (4, 4), F32)
    out = nc.declare_dram_parameter("out", [TOKSLC, DM], F32, isOutput=True)

    with TileContext(nc, pool_alloc_mode="queue") as tc:
        ctx = ExitStack()
        cst = ctx.enter_context(tc.tile_pool(name="cst", bufs=1))
        pers = ctx.enter_context(tc.tile_pool(name="pers", bufs=1))
        scr = ctx.enter_context(tc.tile_pool(name="scr", bufs=2))
        ring = ctx.enter_context(tc.tile_pool(name="ring", bufs=3))
        strm = ctx.enter_context(tc.tile_pool(name="strm", bufs=3))
        ps_acc = ctx.enter_context(tc.tile_pool(name="ps_acc", bufs=2, space="PSUM"))
        ps_gate = ctx.enter_context(tc.tile_pool(name="ps_gate", bufs=1, space="PSUM"))
        ps_mm = ctx.enter_context(tc.tile_pool(name="ps_mm", bufs=3, space="PSUM"))
        ps_tiny = ctx.enter_context(tc.tile_pool(name="ps_tiny", bufs=2, space="PSUM"))
        dram = ctx.enter_context(tc.tile_pool(name="dram", bufs=1, space="DRAM"))

        def cload(pool, pname, shape, dt, rearr=None):
            t = pool.tile(list(shape), dt, name=pname + "_s")
            src = P[pname][:]
            if rearr is not None:
                src = src.rearrange(rearr[0], **rearr[1])
            nc.sync.dma_start(out=t[:], in_=src)
            return t

        wqab_s = cload(cst, "wqab", (128, 8, KH + 2), BF16,
                       ("(kc p) f -> p kc f", dict(p=128)))
        wk_s = cload(cst, "wk", (128, 4, KH), BF16,
                     ("(kc p) f -> p kc f", dict(p=128)))
        wv_s = cload(cst, "wv", (128, 4, VH), BF16,
                     ("(kc p) f -> p kc f", dict(p=128)))
        cqd_s = cload(cst, "cqd", (KH, KS, KH), BF16, ("s p f -> p s f", {}))
        ckd_s = cload(cst, "ckd", (KH, KS, KH), BF16, ("s p f -> p s f", {}))
        cvd_s = cload(cst, "cvd", (KH, KS, 2, KH), BF16, ("s h p f -> p s h f", {}))
        scal_s = cload(cst, "scal", (1, 8), F32)
        png_s = cload(cst, "png", (128, 12), F32,
                      ("(ct p) o -> p (ct o)", dict(p=128)))
        maskS_s = cload(cst, "maskS", (C, C), F32)
        maskI_s = cload(cst, "maskI", (C, C), F32)
        id128f_s = cload(cst, "id128f", (128, 128), F32)
        id128b_s = cload(cst, "id128b", (128, 128), BF16)
        id96b_s = cload(cst, "id96b", (96, 96), BF16)
        id1b_s = cload(cst, "id1b", (1, 1), BF16)
        onesb_s = cload(cst, "onesb", (1, 128), BF16)
        monesb_s = cload(cst, "monesb", (1, 128), BF16)
        one1b_s = cload(cst, "one1b", (1, 1), BF16)
        selq_s = cload(cst, "selq", (B * NCHUNK, 4), F32)
        id4f_s = cload(cst, "id4f", (4, 4), F32)
        ones96b_s = cst.tile([KH, 1], BF16)
        nc.vector.memset(ones96b_s[:], 1.0)
        epsb_s = cst.tile([C, 1], F32)
        nc.vector.memset(epsb_s[:], EPS)
        one1f_s = cst.tile([1, 1], F32)
        nc.vector.memset(one1f_s[:], 1.0)

        # persistent outputs of the scan
        ssqc = pers.tile([C, B * NCHUNK], F32)
        gate = [pers.tile([128, TOKSLC], BF16, name=f"gate{ct}") for ct in range(12)]

        # ---------------- A) projections ----------------
        work_cm = tc.tile_pool(name="work", bufs=1)
        work = work_cm.__enter__()
        rawp_cm = tc.tile_pool(name="rawp", bufs=1)
        rawp = rawp_cm.__enter__()
        qraw = [rawp.tile([KH, 3 + T], BF16, name=f"qraw{p}") for p in range(B)]
        kraw = [rawp.tile([KH, 3 + T], BF16, name=f"kraw{p}") for p in range(B)]
        v0raw = [rawp.tile([KH, 3 + T], BF16, name=f"v0raw{p}") for p in range(B)]
        v1raw = [rawp.tile([KH, 3 + T], BF16, name=f"v1raw{p}") for p in range(B)]
        abd = [dram.tile([2, T], F32, name=f"abd{p}") for p in range(B)]
        for p in range(B):
            for t_ in (qraw[p], kraw[p], v0raw[p], v1raw[p]):
                nc.vector.memset(t_[:, 0:3], 0.0)

        for tt in range(8):
            p, lt = tt // 4, tt % 4
            ps_q = ps_acc.tile([KH + 2, 512], F32, tag="acc")
            for kc in range(8):
                xt = strm.tile([128, 512], BF16, tag="xt")
                nc.gpsimd.dma_start(
                    out=xt[:], in_=xT[kc * 128:(kc + 1) * 128,
                                      tt * 512:(tt + 1) * 512])
                nc.tensor.matmul(ps_q[:], lhsT=wqab_s[:, kc, :],
                                 rhs=xt[:], start=(kc == 0), stop=(kc == 7))
            nc.scalar.copy(out=qraw[p][:, 3 + lt * 512: 3 + (lt + 1) * 512],
                           in_=ps_q[0:KH, :])
            abev = scr.tile([2, 512], F32, tag="abev", bufs=1)
            nc.scalar.copy(out=abev[:], in_=ps_q[KH:KH + 2, :])
            nc.sync.dma_start(out=abd[p][:, lt * 512:(lt + 1) * 512], in_=abev[:])

            ps_k = ps_acc.tile([KH, 512], F32, tag="acc")
            ps_v0 = ps_mm.tile([KH, 512], F32, tag="mm")
            ps_v1 = ps_mm.tile([KH, 512], F32, tag="mm")
            for kc in range(4):
                ct = strm.tile([128, 512], BF16, tag="ct")
                nc.gpsimd.dma_start(
                    out=ct[:], in_=ckvT[kc * 128:(kc + 1) * 128,
                                        tt * 512:(tt + 1) * 512])
                nc.tensor.matmul(ps_k[:], lhsT=wk_s[:, kc, :],
                                 rhs=ct[:], start=(kc == 0), stop=(kc == 3))
                nc.tensor.matmul(ps_v0[:], lhsT=wv_s[:, kc, 0:KH],
                                 rhs=ct[:], start=(kc == 0), stop=(kc == 3))
                nc.tensor.matmul(ps_v1[:], lhsT=wv_s[:, kc, KH:VH],
                                 rhs=ct[:], start=(kc == 0), stop=(kc == 3))
            nc.scalar.copy(out=kraw[p][:, 3 + lt * 512: 3 + (lt + 1) * 512],
                           in_=ps_k[:])
            nc.scalar.copy(out=v0raw[p][:, 3 + lt * 512: 3 + (lt + 1) * 512],
                           in_=ps_v0[:])
            nc.scalar.copy(out=v1raw[p][:, 3 + lt * 512: 3 + (lt + 1) * 512],
                           in_=ps_v1[:])

        # ---------------- B) conv + SiLU (bf16 outs) ----------------
        qn = [work.tile([KH, T], BF16, name=f"qn{p}") for p in range(B)]
        kn = [work.tile([KH, T], BF16, name=f"kn{p}") for p in range(B)]
        vc0 = [work.tile([KH, T], BF16, name=f"vc0{p}") for p in range(B)]
        vc1 = [work.tile([KH, T], BF16, name=f"vc1{p}") for p in range(B)]
        conv_jobs = []
        for p in range(B):
            conv_jobs += [(qraw[p], cqd_s, qn[p], None),
                          (kraw[p], ckd_s, kn[p], None),
                          (v0raw[p], cvd_s, vc0[p], 0),
                          (v1raw[p], cvd_s, vc1[p], 1)]
        for raw, dg, dst, vh in conv_jobs:
            for lt in range(4):
                ps_c = ps_mm.tile([KH, 512], F32, tag="mm")
                for s in range(KS):
                    lhs = dg[:, s, vh, :] if vh is not None else dg[:, s, :]
                    nc.tensor.matmul(ps_c[:], lhsT=lhs,
                                     rhs=raw[:, lt * 512 + s: lt * 512 + s + 512],
                                     start=(s == 0), stop=(s == KS - 1))
                nc.scalar.activation(out=dst[:, lt * 512:(lt + 1) * 512],
                                     in_=ps_c[:], func=AF.Silu)
        rawp_cm.__exit__(None, None, None)

        # gate weights reuse the raw tiles' space (gate GEMM overlaps the scan)
        mid_cm = tc.tile_pool(name="mid", bufs=1)
        mid = mid_cm.__enter__()
        gw_s = [mid.tile([128, VT], BF16, name=f"gwt{kc}") for kc in range(8)]
        xs_s = [mid.tile([128, TOKSLC], BF16, name=f"xst{kc}") for kc in range(8)]
        for kcb in range(8):
            nc.gpsimd.dma_start(out=gw_s[kcb][:],
                                in_=gw[kcb * 128:(kcb + 1) * 128, :])
            nc.gpsimd.dma_start(out=xs_s[kcb][:],
                                in_=xsT[kcb * 128:(kcb + 1) * 128, :])

        # ---------------- C) L2 norm of q,k (in place) ----------------
        for p in range(B):
            for src, qscale in ((qn[p], KH ** -0.5), (kn[p], None)):
                for lt in range(4):
                    sl = slice(lt * 512, (lt + 1) * 512)
                    sq = scr.tile([KH, 512], BF16, tag="sq")
                    nc.vector.tensor_tensor(out=sq[:], in0=src[:, sl],
                                            in1=src[:, sl], op=AL.mult)
                    ps_n = ps_tiny.tile([1, 512], F32, tag="tiny")
                    nc.tensor.matmul(ps_n[:], lhsT=ones96b_s[:], rhs=sq[:],
                                     start=True, stop=True)
                    # factor = qscale/sqrt(ssq) = rsqrt(ssq/qscale^2): one ACT
                    # Rsqrt (eps negligible: ||q|| >> 1e-6 for silu-conv outs).
                    nrb = scr.tile([1, 512], BF16, tag="recb")
                    iscale = (1.0 / float(qscale) ** 2 if qscale is not None
                              else 1.0)
                    eng = nc.scalar
                    eng.add_instruction(mybir.InstActivation(
                        name=nc.get_next_instruction_name(),
                        func=AF.Rsqrt,
                        ins=[eng.lower_ap(ps_n[:]),
                             mybir.ImmediateValue(dtype=F32, value=0.0),
                             mybir.ImmediateValue(dtype=F32, value=iscale),
                             mybir.ImmediateValue(dtype=F32, value=0.0)],
                        outs=[eng.lower_ap(nrb[:])]))
                    ps_f = ps_mm.tile([KH, 512], F32, tag="mm")
                    nc.tensor.matmul(ps_f[:], lhsT=onesb_s[0:1, 0:KH],
                                     rhs=nrb[:], start=True, stop=True)
                    nc.vector.tensor_tensor(out=src[:, sl], in0=src[:, sl],
                                            in1=ps_f[:], op=AL.mult)

        # ---------------- D) decay rows + cumsum (bf16 hi/lo rows) -----------
        Lh = [work.tile([1, T], BF16, name=f"Lh{p}") for p in range(B)]
        Ll = [work.tile([1, T], BF16, name=f"Ll{p}") for p in range(B)]
        brow = [work.tile([1, T], BF16, name=f"brow{p}") for p in range(B)]
        lnbrow = [work.tile([1, T], BF16, name=f"lnbrow{p}") for p in range(B)]
        for p in range(B):
            arow = scr.tile([1, T], F32, tag="rowC", bufs=1)
            nc.sync.dma_start(out=arow[:], in_=abd[p][0:1, :])
            rsc0 = scr.tile([1, T], F32, tag="rowD", bufs=1)
            nc.scalar.activation(out=rsc0[:], in_=arow[:], func=AF.Exp,
                                 bias=scal_s[0:1, 0:1], scale=1.0)
            rsc = scr.tile([1, T], F32, tag="rowC", bufs=1)
            nc.scalar.activation(out=rsc[:], in_=rsc0[:], func=AF.Ln,
                                 bias=one1f_s[:], scale=1.0)
            brawrow = scr.tile([1, T], F32, tag="rowC", bufs=1)
            nc.sync.dma_start(out=brawrow[:], in_=abd[p][1:2, :])
            grow = scr.tile([1, T], F32, tag="rowB", bufs=1)
            nc.vector.tensor_scalar(out=grow[:], in0=rsc[:],
                                    scalar1=scal_s[0:1, 1:2], scalar2=None,
                                    op0=AL.mult)
            nc.scalar.activation(out=brow[p][:], in_=brawrow[:],
                                 func=AF.Sigmoid, bias=scal_s[0:1, 2:3], scale=1.0)
            lnb_e = scr.tile([1, T], F32, tag="rowD", bufs=1)
            nc.scalar.activation(out=lnb_e[:], in_=brawrow[:], func=AF.Exp,
                                 bias=scal_s[0:1, 3:4], scale=-1.0)
            lnb_t = scr.tile([1, T], F32, tag="rowC", bufs=1)
            nc.scalar.activation(out=lnb_t[:], in_=lnb_e[:], func=AF.Ln,
                                 bias=one1f_s[:], scale=1.0)
            nc.vector.tensor_scalar_mul(out=lnbrow[p][:], in0=lnb_t[:], scalar1=-1.0)
            gb = dram.tile([NCHUNK, C], F32, name=f"gb{p}")
            nc.sync.dma_start(
                out=gb[:].rearrange("p f -> (p f)").unsqueeze(0), in_=grow[:])
            g16 = scr.tile([NCHUNK, C], F32, tag="g16")
            nc.sync.dma_start(out=g16[:], in_=gb[:])
            L16t = scr.tile([NCHUNK, C], F32, tag="L16t")
            nc.vector.tensor_tensor_scan(out=L16t[:], data0=g16[:], data1=g16[:],
                                         initial=0.0, op0=AL.add, op1=AL.bypass)
            gb2 = dram.tile([NCHUNK, C], F32, name=f"gb2{p}")
            nc.sync.dma_start(out=gb2[:], in_=L16t[:])
            Lrow = scr.tile([1, T], F32, tag="rowC", bufs=1)
            nc.sync.dma_start(
                out=Lrow[:], in_=gb2[:].rearrange("p f -> (p f)").unsqueeze(0))
            nc.vector.tensor_copy(out=Lh[p][:], in_=Lrow[:])
            nc.vector.tensor_tensor(out=Ll[p][:], in0=Lrow[:], in1=Lh[p][:],
                                    op=AL.subtract)

        # -------- E/F/G) per-chunk prep + sequential sweep + output ----------
        a2ain = [dram.tile([8, VH, TOKSLC // 2], BF16, name=f"a2ain{h_}")
                 for h_ in range(2)]
        a2aout = [dram.tile([8, VH, TOKSLC // 2], BF16, name=f"a2aout{h_}")
                  for h_ in range(2)]
        Scur = []
        for p in range(B):
            s0 = ring.tile([KH, VH], BF16, tag=f"Sh{p}", bufs=4, name=f"S0_{p}")
            nc.vector.memset(s0[:], 0.0)
            Scur.append(s0)

        for i in range(NCHUNK):
            for p in range(B):
                j = i * B + p
                ck = slice(i * C, (i + 1) * C)
                lstc = slice((i + 1) * C - 1, (i + 1) * C)
                Lrh, Lrl = Lh[p][0:1, ck], Ll[p][0:1, ck]

                # E[t,s] = L_t - L_s via 4 rank-1 bf16 matmuls (hi/lo
                # exact); E, Gkk, Gqk packed into one psum bank.
                ps_egg = ps_mm.tile([C, 3, C], F32, tag="mm")
                ps_e = ps_egg[:, 0]
                nc.tensor.matmul(ps_e, lhsT=Lrh, rhs=onesb_s[:],
                                 start=True, stop=False)
                nc.tensor.matmul(ps_e, lhsT=Lrl, rhs=onesb_s[:],
                                 start=False, stop=False)
                nc.tensor.matmul(ps_e, lhsT=monesb_s[:], rhs=Lrh,
                                 start=False, stop=False)
                nc.tensor.matmul(ps_e, lhsT=monesb_s[:], rhs=Lrl,
                                 start=False, stop=True)
                Es = scr.tile([C, C], F32, tag="Es", bufs=3)
                nc.vector.tensor_tensor(out=Es[:], in0=ps_e, in1=maskS_s[:],
                                        op=AL.add)
                Ei = scr.tile([C, C], F32, tag="Ei", bufs=3)
                nc.vector.tensor_tensor(out=Ei[:], in0=ps_e, in1=maskI_s[:],
                                        op=AL.add)

                # lnb / beta columns
                ps_lb = ps_tiny.tile([C, 4], BF16, tag="tiny")
                nc.tensor.transpose(ps_lb[:, 0:1], lnbrow[p][0:1, ck], id1b_s[:])
                nc.tensor.transpose(ps_lb[:, 2:3], brow[p][0:1, ck], id1b_s[:])
                lnbc = scr.tile([C, 1], F32, tag="lnbc")
                nc.scalar.copy(out=lnbc[:], in_=ps_lb[:, 0:1])
                bc = scr.tile([C, 1], F32, tag="bc")
                nc.scalar.copy(out=bc[:], in_=ps_lb[:, 2:3])

                Mexp = scr.tile([C, C], F32, tag="Mexp", bufs=3)
                nc.scalar.activation(out=Mexp[:], in_=Es[:], func=AF.Exp,
                                     bias=lnbc[:], scale=1.0)
                Dincl = scr.tile([C, C], F32, tag="Dincl", bufs=3)
                nc.scalar.activation(out=Dincl[:], in_=Ei[:], func=AF.Exp)

                # N (strict-lower, includes beta) and its transpose
                ps_gk = ps_egg[:, 1]
                nc.tensor.matmul(ps_gk, lhsT=kn[p][:, ck], rhs=kn[p][:, ck],
                                 start=True, stop=True)
                Nbf = scr.tile([C, C], BF16, tag="Nbf", bufs=3)
                nc.vector.scalar_tensor_tensor(out=Nbf[:], in0=ps_gk,
                                               scalar=-1.0, in1=Mexp[:],
                                               op0=AL.mult, op1=AL.mult)
                # P = (q_t . k_s) * Dincl
                ps_gq = ps_egg[:, 2]
                nc.tensor.matmul(ps_gq, lhsT=qn[p][:, ck], rhs=kn[p][:, ck],
                                 start=True, stop=True)
                Pbf = scr.tile([C, C], BF16, tag="Pbf", bufs=3)
                nc.vector.tensor_tensor(out=Pbf[:], in0=ps_gq, in1=Dincl[:],
                                        op=AL.mult)
                # both transposes (N, P) share one psum bank
                ps_nt = ps_mm.tile([C, 2, C], BF16, tag="mm")
                nc.tensor.transpose(ps_nt[:, 0], Nbf[:], id128b_s[:])
                nc.tensor.transpose(ps_nt[:, 1], Pbf[:], id128b_s[:])
                ev = nc.scalar.copy if (j % 2 == 0) else \
                    (lambda out, in_: nc.vector.tensor_copy(out=out, in_=in_))
                NTbf = scr.tile([C, C], BF16, tag="NTbf", bufs=3)
                ev(out=NTbf[:], in_=ps_nt[:, 0])
                P0b = scr.tile([C, C], BF16, tag="P0b", bufs=3)
                nc.vector.tensor_tensor(out=P0b[:], in0=ps_nt[:, 0],
                                        in1=id128f_s[:], op=AL.add)
                PTb = ring.tile([C, C], BF16, tag="PTb", name=f"PTb{j}")
                nc.scalar.copy(out=PTb[:], in_=ps_nt[:, 1])

                # doubling powers (levels=3)
                ps_sq = ps_mm.tile([C, 2, C], F32, tag="mm")
                nc.tensor.matmul(ps_sq[:, 0], lhsT=NTbf[:], rhs=Nbf[:],
                                 start=True, stop=True)
                nc.tensor.matmul(ps_sq[:, 1], lhsT=Nbf[:], rhs=NTbf[:],
                                 start=True, stop=True)
                N2r = scr.tile([C, C], BF16, tag="N2r", bufs=3)
                ev(out=N2r[:], in_=ps_sq[:, 0])
                N2i = scr.tile([C, C], BF16, tag="N2i", bufs=3)
                nc.vector.tensor_tensor(out=N2i[:], in0=ps_sq[:, 0],
                                        in1=id128f_s[:], op=AL.add)
                N2Tr = scr.tile([C, C], BF16, tag="N2Tr", bufs=3)
                ev(out=N2Tr[:], in_=ps_sq[:, 1])
                ps_sq3 = ps_mm.tile([C, 2, C], F32, tag="mm")
                nc.tensor.matmul(ps_sq3[:, 0], lhsT=N2Tr[:], rhs=N2r[:],
                                 start=True, stop=True)
                nc.tensor.matmul(ps_sq3[:, 1], lhsT=N2r[:], rhs=N2Tr[:],
                                 start=True, stop=True)
                N4r = scr.tile([C, C], BF16, tag="N4r", bufs=3)
                ev(out=N4r[:], in_=ps_sq3[:, 0])
                N4i = scr.tile([C, C], BF16, tag="N4i", bufs=3)
                nc.vector.tensor_tensor(out=N4i[:], in0=ps_sq3[:, 0],
                                        in1=id128f_s[:], op=AL.add)
                N4Tr = scr.tile([C, C], BF16, tag="N4Tr", bufs=3)
                ev(out=N4Tr[:], in_=ps_sq3[:, 1])
                ps_sq5 = ps_mm.tile([C, C], F32, tag="mm")
                nc.tensor.matmul(ps_sq5[:], lhsT=N4Tr[:], rhs=N4r[:],
                                 start=True, stop=True)
                N8i = scr.tile([C, C], BF16, tag="N8i", bufs=3)
                nc.vector.tensor_tensor(out=N8i[:], in0=ps_sq5[:], in1=id128f_s[:],
                                        op=AL.add)

                # chain: T^T = (I+N8T)(I+N4T)(I+N2T)(I+NT)
                ps_c1 = ps_mm.tile([C, C], F32, tag="mm")
                nc.tensor.matmul(ps_c1[:], lhsT=N2i[:], rhs=P0b[:],
                                 start=True, stop=True)
                C1 = scr.tile([C, C], BF16, tag="C1", bufs=3)
                ev(out=C1[:], in_=ps_c1[:])
                ps_c2 = ps_mm.tile([C, C], F32, tag="mm")
                nc.tensor.matmul(ps_c2[:], lhsT=N4i[:], rhs=C1[:],
                                 start=True, stop=True)
                C2 = scr.tile([C, C], BF16, tag="C2", bufs=3)
                ev(out=C2[:], in_=ps_c2[:])
                ps_c3 = ps_mm.tile([C, C], F32, tag="mm")
                nc.tensor.matmul(ps_c3[:], lhsT=N8i[:], rhs=C2[:],
                                 start=True, stop=True)
                TTm = ring.tile([C, C], BF16, tag="TTm", name=f"TTm{j}")
                nc.scalar.copy(out=TTm[:], in_=ps_c3[:])

                # V halves + K token-layout transposes share one psum bank
                bV = ring.tile([C, VH], BF16, tag="bV", name=f"bV{j}")
                ps_vt = ps_mm.tile([C, 3, KH], BF16, tag="mm")
                nc.tensor.transpose(ps_vt[:, 0], vc0[p][:, ck], id96b_s[:])
                nc.tensor.transpose(ps_vt[:, 1], vc1[p][:, ck], id96b_s[:])
                nc.tensor.transpose(ps_vt[:, 2], kn[p][:, ck], id96b_s[:])
                for hh in range(2):
                    nc.vector.tensor_scalar(out=bV[:, hh * KH:(hh + 1) * KH],
                                            in0=ps_vt[:, hh], scalar1=bc[:],
                                            scalar2=None, op0=AL.mult)
                ps_kt = ps_vt[:, 2]
                ps_ll3 = ps_tiny.tile([C, 3], F32, tag="tiny")
                nc.tensor.matmul(ps_ll3[:, 0:1], lhsT=onesb_s[:],
                                 rhs=Lh[p][0:1, lstc], start=True, stop=False)
                nc.tensor.matmul(ps_ll3[:, 0:1], lhsT=onesb_s[:],
                                 rhs=Ll[p][0:1, lstc], start=False, stop=True)
                Llc = scr.tile([C, 1], F32, tag="Llc")
                nc.scalar.copy(out=Llc[:], in_=ps_ll3[:, 0:1])
                nc.tensor.matmul(ps_ll3[:, 1:2], lhsT=Lrh, rhs=one1b_s[:],
                                 start=True, stop=False)
                nc.tensor.matmul(ps_ll3[:, 1:2], lhsT=Lrl, rhs=one1b_s[:],
                                 start=False, stop=True)
                eLl = scr.tile([C, 1], F32, tag="eLl")
                nc.scalar.activation(out=eLl[:], in_=ps_ll3[:, 1:2], func=AF.Exp,
                                     bias=Llc[:], scale=-1.0)
                Ktok = ring.tile([C, KH], BF16, tag="Ktok", name=f"Ktok{j}")
                nc.vector.tensor_scalar(out=Ktok[:], in0=ps_kt, scalar1=eLl[:],
                                        scalar2=None, op0=AL.mult)

                # A_t column, beta*A column, chunk-total decay
                Acol = ring.tile([C, 1], F32, tag="Acol", name=f"Acol{j}")
                nc.scalar.activation(out=Acol[:], in_=ps_ll3[:, 1:2], func=AF.Exp)
                bAcol = ring.tile([C, 1], F32, tag="bAcol", name=f"bAcol{j}")
                nc.vector.tensor_tensor(out=bAcol[:], in0=Acol[:], in1=bc[:],
                                        op=AL.mult)
                nc.tensor.matmul(ps_ll3[0:KH, 2:3], lhsT=onesb_s[0:1, 0:KH],
                                 rhs=Lh[p][0:1, lstc], start=True, stop=False)
                nc.tensor.matmul(ps_ll3[0:KH, 2:3], lhsT=onesb_s[0:1, 0:KH],
                                 rhs=Ll[p][0:1, lstc], start=False, stop=True)
                aC96 = ring.tile([KH, 1], F32, tag="aC96", name=f"aC96{j}")
                nc.scalar.activation(out=aC96[:], in_=ps_ll3[0:KH, 2:3],
                                     func=AF.Exp)

                # ---- sequential sweep step ----
                ps_y = ps_mm.tile([C, VH], F32, tag="mm")
                nc.tensor.matmul(ps_y[:], lhsT=kn[p][:, ck], rhs=Scur[p][:],
                                 start=True, stop=True)
                R2 = scr.tile([C, VH], BF16, tag="R2")
                nc.vector.scalar_tensor_tensor(out=R2[:], in0=ps_y[:],
                                               scalar=bAcol[:], in1=bV[:],
                                               op0=AL.mult, op1=AL.subtract)
                ps_u = ps_mm.tile([C, VH], F32, tag="mm")
                nc.tensor.matmul(ps_u[:], lhsT=TTm[:], rhs=R2[:],
                                 start=True, stop=True)
                U = ring.tile([C, VH], BF16, tag="U", name=f"U{j}")
                nc.scalar.activation(out=U[:], in_=ps_u[:], func=AF.Copy,
                                     scale=-1.0)
                ps_s = ps_mm.tile([KH, VH], F32, tag="mm")
                nc.tensor.matmul(ps_s[:], lhsT=Ktok[:], rhs=U[:],
                                 start=True, stop=True)
                Snew = ring.tile([KH, VH], BF16, tag=f"Sh{p}", bufs=4,
                                 name=f"S{p}_{i + 1}")
                nc.vector.scalar_tensor_tensor(out=Snew[:], in0=Scur[p][:],
                                               scalar=aC96[:], in1=ps_s[:],
                                               op0=AL.mult, op1=AL.add)

                # ---- output epilogue ----
                ps_pu = ps_acc.tile([C, VH], F32, tag="acc")
                nc.tensor.matmul(ps_pu[:], lhsT=PTb[:], rhs=U[:],
                                 start=True, stop=True)
                ps_z = ps_acc.tile([C, VH], F32, tag="acc")
                nc.tensor.matmul(ps_z[:], lhsT=qn[p][:, ck], rhs=Scur[p][:],
                                 start=True, stop=True)
                tmpZ = scr.tile([C, VH], F32, tag="tmpZ")
                nc.vector.tensor_scalar(out=tmpZ[:], in0=ps_z[:], scalar1=Acol[:],
                                        scalar2=None, op0=AL.mult)
                Osb = scr.tile([C, VH], BF16, tag="Osb", bufs=3)
                nc.vector.tensor_tensor(out=Osb[:], in0=ps_pu[:], in1=tmpZ[:],
                                        op=AL.add)
                sqo = scr.tile([C, VH], BF16, tag="sqo")
                nc.vector.scalar_tensor_tensor(out=sqo[:], in0=Osb[:],
                                               scalar=1.0, in1=Osb[:],
                                               op0=AL.mult, op1=AL.mult,
                                               accum_out=ssqc[:, j:j + 1])
                # transpose + ship (unnormalized) o to the A2A send buffer;
                # the RMS factor is applied per-token at the final GEMM evict.
                tok0 = p * T + i * C
                d, off = tok0 // TOKSLC, tok0 % TOKSLC
                half, off2 = (0, off) if off < TOKSLC // 2 else (1, off - TOKSLC // 2)
                ps_ot = ps_mm.tile([KH, 2, C], BF16, tag="mm")
                nc.tensor.transpose(ps_ot[:, 0], Osb[:, 0:KH], id128b_s[:])
                nc.tensor.transpose(ps_ot[:, 1], Osb[:, KH:VH], id128b_s[:])
                for hh in range(2):
                    otb = scr.tile([KH, C], BF16, tag="otb")
                    ev(out=otb[:], in_=ps_ot[:, hh])
                    nc.sync.dma_start(
                        out=a2ain[half][d, hh * KH:(hh + 1) * KH, off2:off2 + C],
                        in_=otb[:])
                Scur[p] = Snew
                if i == NCHUNK - 3 and p == B - 1:
                    # all chunks of the first token-half are now emitted
                    nc.gpsimd.collective_compute(
                        "AllToAll", AL.bypass, replica_groups=[list(range(8))],
                        ins=[a2ain[0].opt()], outs=[a2aout[0].opt()])

        # ---------------- K) gate GEMM (overlaps scan via dataflow) ----------
        for ctb in range(12):
            ps_g = ps_gate.tile([128, TOKSLC], F32, tag="gate")
            for kcb in range(8):
                nc.tensor.matmul(ps_g[:],
                                 lhsT=gw_s[kcb][:, ctb * 128:(ctb + 1) * 128],
                                 rhs=xs_s[kcb][:], start=(kcb == 0), stop=(kcb == 7))
            nc.scalar.activation(out=gate[ctb][:], in_=ps_g[:], func=AF.Silu)

        # ---------------- H) ssq AllReduce + rsqrt ----------------
        arin = dram.tile([C, B * NCHUNK], F32, name="arin")
        arout = dram.tile([C, B * NCHUNK], F32, name="arout")
        nc.sync.dma_start(out=arin[:], in_=ssqc[:])
        nc.gpsimd.collective_compute(
            "AllReduce", AL.add, replica_groups=[list(range(8))],
            ins=[arin.opt()], outs=[arout.opt()])
        rq = pers.tile([C, B * NCHUNK], F32)
        nc.sync.dma_start(out=rq[:], in_=arout[:])
        rb = pers.tile([C, B * NCHUNK], F32)
        nc.scalar.activation(out=rb[:], in_=rq[:], func=AF.Sqrt,
                             bias=epsb_s[:], scale=1.0 / VT)
        rs = pers.tile([C, B * NCHUNK], F32)
        nc.vector.reciprocal(out=rs[:], in_=rb[:])
        # gather this core's 4 per-token-block rs columns via one-hot matmul
        ps_rt = ps_tiny.tile([B * NCHUNK, C], F32, tag="tiny")
        nc.tensor.transpose(ps_rt[:], rs[:], id128f_s[:])
        rsT = pers.tile([B * NCHUNK, C], F32)
        nc.scalar.copy(out=rsT[:], in_=ps_rt[:])
        ps_r4 = ps_tiny.tile([4, C], F32, tag="tiny")
        nc.tensor.matmul(ps_r4[:], lhsT=selq_s[:], rhs=rsT[:],
                         start=True, stop=True)
        rs4T = pers.tile([4, C], F32)
        nc.scalar.copy(out=rs4T[:], in_=ps_r4[:])
        ps_rq = ps_tiny.tile([C, 4], F32, tag="tiny")
        nc.tensor.transpose(ps_rq[:], rs4T[:], id4f_s[:])
        rsq_sb = pers.tile([C, 4], F32)
        nc.scalar.copy(out=rsq_sb[:], in_=ps_rq[:])
        rsq = [rsq_sb[:, to:to + 1] for to in range(4)]

        # second-half A2A (first half fires inside the chunk loop)
        nc.gpsimd.collective_compute(
            "AllToAll", AL.bypass, replica_groups=[list(range(8))],
            ins=[a2ain[1].opt()], outs=[a2aout[1].opt()])

        mid_cm.__exit__(None, None, None)
        work_cm.__exit__(None, None, None)

        # ---------------- L) OG product + final GEMM ----------------
        late_cm = tc.tile_pool(name="late", bufs=1)
        late = late_cm.__enter__()
        wo_s = [late.tile([128, DM], BF16, name=f"wot{ct}") for ct in range(12)]
        for ct in range(12):
            nc.gpsimd.dma_start(out=wo_s[ct][:],
                                in_=wo[ct * 128:(ct + 1) * 128, :])
        og = [late.tile([128, TOKSLC], BF16, name=f"og{ct}") for ct in range(12)]
        HT = TOKSLC // 2
        flats = [a2aout[h_][:].rearrange("h c t -> (h c) t") for h_ in range(2)]
        for half in range(2):
            for ct in range(12):
                ogin = late.tile([128, HT], BF16, tag="ogin", bufs=3,
                                 name=f"ogin{half}_{ct}")
                nc.gpsimd.dma_start(out=ogin[:],
                                    in_=flats[half][ct * 128:(ct + 1) * 128, :])
                nc.vector.scalar_tensor_tensor(
                    out=og[ct][:, half * HT:(half + 1) * HT], in0=ogin[:],
                    scalar=png_s[:, ct:ct + 1],
                    in1=gate[ct][:, half * HT:(half + 1) * HT],
                    op0=AL.mult, op1=AL.mult)
        for to in range(4):
            # rs column for this 128-token block of this core's slice
            # (core id is data-independent: token block -> (p, i) -> ssqc col)
            for fo in range(2):
                ps_o = ps_gate.tile([128, 512], F32, tag="gate")
                for ct in range(12):
                    nc.tensor.matmul(ps_o[:],
                                     lhsT=og[ct][:, to * 128:(to + 1) * 128],
                                     rhs=wo_s[ct][:, fo * 512:(fo + 1) * 512],
                                     start=(ct == 0), stop=(ct == 11))
                osb = late.tile([128, 512], F32, tag="osb", bufs=2,
                                name=f"osb{to}_{fo}")
                nc.vector.tensor_scalar(out=osb[:], in0=ps_o[:],
                                        scalar1=rsq[to][:], scalar2=None,
                                        op0=AL.mult)
                nc.sync.dma_start(
                    out=out[to * 128:(to + 1) * 128, fo * 512:(fo + 1) * 512],
                    in_=osb[:])
        late_cm.__exit__(None, None, None)
        ctx.close()

    return nc


def kernel(x, c_kv, w_q, w_k, w_v, conv_q_w, conv_q_b, conv_k_w, conv_k_b,
           conv_v_w, conv_v_b, a_proj_w, a_proj_b, A_log, dt_bias,
           b_proj_w, b_proj_b, g_proj_w, post_norm_w, w_o):
    from concourse.bass_utils import run_bass_kernel_spmd

    bf = ml_dtypes.bfloat16
    x = np.asarray(x, np.float32)
    c_kv = np.asarray(c_kv, np.float32)
    xT = np.ascontiguousarray(x.reshape(NTOK, DM).T).astype(bf)
    ckvT = np.ascontiguousarray(c_kv.reshape(NTOK, DKV).T).astype(bf)
    gw = np.asarray(g_proj_w, np.float32).astype(bf)
    wo_ = np.asarray(w_o, np.float32).astype(bf)
    png = np.asarray(post_norm_w, np.float32).reshape(VT, 1)

    maskS = np.where(np.arange(C)[None, :] < np.arange(C)[:, None], 0.0, NEG)
    maskI = np.where(np.arange(C)[None, :] <= np.arange(C)[:, None], 0.0, NEG)
    consts = dict(
        maskS=maskS.astype(np.float32), maskI=maskI.astype(np.float32),
        id128f=np.eye(128, dtype=np.float32),
        id128b=np.eye(128, dtype=np.float32).astype(bf),
        id96b=np.eye(96, dtype=np.float32).astype(bf),
        id1b=np.ones((1, 1), np.float32).astype(bf),
        onesb=np.ones((1, 128), np.float32).astype(bf),
        monesb=(-np.ones((1, 128), np.float32)).astype(bf),
        one1b=np.ones((1, 1), np.float32).astype(bf),
        id4f=np.eye(4, dtype=np.float32),
    )

    in_maps = []
    for c in range(8):
        h = c
        qs = slice(h * KH, (h + 1) * KH)
        vs = slice(h * VH, (h + 1) * VH)
        wqab_ = np.concatenate([
            np.asarray(w_q, np.float32)[:, qs],
            np.asarray(a_proj_w, np.float32)[:, h:h + 1],
            np.asarray(b_proj_w, np.float32)[:, h:h + 1]], axis=1).astype(bf)
        cq = np.asarray(conv_q_w, np.float32)[qs, 0, :]
        ck = np.asarray(conv_k_w, np.float32)[qs, 0, :]
        cv = np.asarray(conv_v_w, np.float32)[vs, 0, :]
        cqd_ = np.stack([np.diag(cq[:, s]) for s in range(KS)]).astype(bf)
        ckd_ = np.stack([np.diag(ck[:, s]) for s in range(KS)]).astype(bf)
        cvd_ = np.stack([np.stack([np.diag(cv[hh * KH:(hh + 1) * KH, s])
                                   for hh in range(2)])
                         for s in range(KS)]).astype(bf)
        scal_ = np.zeros((1, 8), np.float32)
        scal_[0, 0] = float(np.asarray(dt_bias)[h] + np.asarray(a_proj_b)[h])
        scal_[0, 1] = -float(np.exp(np.asarray(A_log)[h]))
        scal_[0, 2] = float(np.asarray(b_proj_b)[h])
        scal_[0, 3] = -float(np.asarray(b_proj_b)[h])
        selq = np.zeros((B * NCHUNK, 4), np.float32)
        for to in range(4):
            tok0 = c * TOKSLC + to * 128
            p_, i_ = tok0 // T, (tok0 % T) // C
            selq[i_ * B + p_, to] = 1.0
        m = dict(
            selq=selq,
            xT=xT, ckvT=ckvT, wqab=wqab_,
            wk=np.asarray(w_k, np.float32)[:, qs].astype(bf),
            wv=np.asarray(w_v, np.float32)[:, vs].astype(bf),
            cqd=cqd_, ckd=ckd_, cvd=cvd_, scal=scal_, gw=gw, wo=wo_,
            xsT=np.ascontiguousarray(xT[:, c * TOKSLC:(c + 1) * TOKSLC]),
            png=png, **consts)
        in_maps.append(m)

    if "nc" not in _CACHE:
        _CACHE["nc"] = _build()
    res = run_bass_kernel_spmd(_CACHE["nc"], in_maps, core_ids=list(range(8)))
    _CACHE["last"] = res
    parts = [np.asarray(res.results[c]["out"], np.float32) for c in range(8)]
    return np.concatenate(parts, axis=0).reshape(B, T, DM)


# revision 27
# speedup vs baseline: 1.0202x; 1.0039x over previous
"""nn_GatedDeltaRecurrence Trainium2 kernel (8 NeuronCores, Bass/Tile).

Sharding: core c owns head h=c for both batches (16 (b,h) pairs / 8 cores = 2
pairs per core: data-parallel B x tensor-parallel H per the spec hint). Each
core computes its head's q/k/v/a/b projections + short-conv + norms from the
(host-staged) full inputs, runs the gated delta recurrence in chunked form
(C=128, UT transform, truncated-doubling triangular solve, levels=3), then the
cores exchange per-head outputs with an AllToAll so each core finishes
token-parallel (RMS norm + gate + output projection) for its 512-token slice.
A 16KB AllReduce carries the cross-head sum-of-squares for the RMS norm.

Matmuls run in bf16 with fp32 PSUM accumulation (validated absmax/scale ~5e-3
vs the fp32 reference on this problem's data, well under the 2e-2 gate). The
within-chunk decay exponents L are kept to full fp32 precision on the PE by
splitting them into bf16 hi+lo parts and accumulating rank-1 matmuls.

This build works around a walrus codegen limitation in this container
(instructions with >1 sync-wait commands are rejected) by splitting waits
onto same-engine NOPs at Tile commit time.
"""
import sys

sys.path.insert(0, "/opt/trn_rl_repo")
sys.path.insert(0, "/opt/pypackages")

import numpy as np
import ml_dtypes

B, T = 2, 2048
DM, DKV = 1024, 512
H, KH, VH = 8, 96, 192
KQT, VT = H * KH, H * VH
KS = 4
EPS = 1e-6
C = 128                      # chunk length
NCHUNK = T // C              # 16 chunks per pair
NTOK = B * T                 # 4096 tokens
TOKSLC = NTOK // 8           # 512 tokens per core in the final phase
NEG = -30000.0               # additive mask (exp -> 0)

_CACHE = {}


def _build():
    import bass_rust
    from contextlib import ExitStack
    from concourse import bass, mybir
    from concourse.tile import TileContext
    from concourse.vector_clock import ScopedClock

    F32, BF16 = mybir.dt.float32, mybir.dt.bfloat16
    AL = mybir.AluOpType
    AF = mybir.ActivationFunctionType

    # ---- workarounds: walrus rejects >1 sync-wait per instruction ----
    def _drain_patch(self, tick_clock, wait_clock):
        carrier = self.nc.sync.nop(nofuse=True, hint="drain_waits")
        wait_clock.add_sem_waits(
            carrier.ins, ScopedClock({None: tick_clock.global_clock}))
        si = carrier.ins.sync_info
        waits = list(si.on_wait) if si is not None else []
        if len(waits) > 1:
            carrier.ins.sync_info = bass_rust.SyncInfo(
                on_wait=[waits[0]], on_update=[])
            for w in waits[1:]:
                extra = self.nc.sync.nop(nofuse=True, hint="drain_waits")
                extra.ins.sync_info = bass_rust.SyncInfo(
                    on_wait=[w], on_update=[])
        self.nc.sync.drain()
        self.nc.all_engine_barrier()
        popped = self.nc._tile_sem_poison_stack.pop()
        assert popped is self._sem_poison
        self.nc.clear_and_free_semaphores(
            list(self.sems.allocated().values()))
        self.nc.all_engine_barrier()

    TileContext._drain_and_barrier = _drain_patch
    if not getattr(TileContext, "_split_waits_patched", False):
        _orig_commit = TileContext._commit_instruction

        def _commit_split(self, inst, lazy_reg_writes=True):
            si = getattr(inst, "sync_info", None)
            if (si is not None and si.on_wait
                    and inst.engine != mybir.EngineType.Unassigned
                    and (len(si.on_wait) > 1
                         or isinstance(inst, mybir.InstDrain))):
                waits = list(si.on_wait)
                keep = [] if isinstance(inst, mybir.InstDrain) else [waits.pop(0)]
                for w in waits:
                    nop = mybir.InstNoOp(
                        name=self.nc.get_next_instruction_name(),
                        engine=inst.engine, ins=[], outs=[], debug=inst.debug)
                    nop.sync_info = bass_rust.SyncInfo(on_wait=[w], on_update=[])
                    self.nc.register_instruction(nop, overwrite=True)
                    self._add_instruction(nop)
                inst.sync_info = bass_rust.SyncInfo(
                    on_wait=keep, on_update=list(si.on_update))
            return _orig_commit(self, inst, lazy_reg_writes)

        TileContext._commit_instruction = _commit_split
        TileContext._split_waits_patched = True

    nc = bass.Bass()
    P = {}

    def dp(name, shape, dt):
        P[name] = nc.declare_dram_parameter(name, list(shape), dt, isOutput=False)
        return P[name]

    xT = dp("xT", (DM, NTOK), BF16)
    ckvT = dp("ckvT", (DKV, NTOK), BF16)
    wqab = dp("wqab", (DM, KH + 2), BF16)
    wk = dp("wk", (DKV, KH), BF16)
    wv = dp("wv", (DKV, VH), BF16)
    cqd = dp("cqd", (KS, KH, KH), BF16)
    ckd = dp("ckd", (KS, KH, KH), BF16)
    cvd = dp("cvd", (KS, 2, KH, KH), BF16)
    scal = dp("scal", (1, 8), F32)
    gw = dp("gw", (DM, VT), BF16)
    wo = dp("wo", (VT, DM), BF16)
    xsT = dp("xsT", (DM, TOKSLC), BF16)
    png = dp("png", (VT, 1), F32)
    maskS = dp("maskS", (C, C), F32)
    maskI = dp("maskI", (C, C), F32)
    id128f = dp("id128f", (128, 128), F32)
    id128b = dp("id128b", (128, 128), BF16)
    id96b = dp("id96b", (96, 96), BF16)
    id1b = dp("id1b", (1, 1), BF16)
    onesb = dp("onesb", (1, 128), BF16)
    monesb = dp("monesb", (1, 128), BF16)
    one1b = dp("one1b", (1, 1), BF16)
    selq = dp("selq", (B * NCHUNK, 4), F32)
    ones2b = dp("ones2b", (2, 128), BF16)
    mones2b = dp("mones2b", (2, 128), BF16)
    one2b = dp("one2b", (2, 1), BF16)
    id4f = dp("id4f", (4, 4), F32)
    out = nc.declare_dram_parameter("out", [TOKSLC, DM], F32, isOutput=True)

    with TileContext(nc, pool_alloc_mode="queue") as tc:
        ctx = ExitStack()
        cst = ctx.enter_context(tc.tile_pool(name="cst", bufs=1))
        pers = ctx.enter_context(tc.tile_pool(name="pers", bufs=1))
        scr = ctx.enter_context(tc.tile_pool(name="scr", bufs=2))
        ring = ctx.enter_context(tc.tile_pool(name="ring", bufs=3))
        strm = ctx.enter_context(tc.tile_pool(name="strm", bufs=3))
        ps_acc = ctx.enter_context(tc.tile_pool(name="ps_acc", bufs=2, space="PSUM"))
        ps_gate = ctx.enter_context(tc.tile_pool(name="ps_gate", bufs=1, space="PSUM"))
        ps_mm = ctx.enter_context(tc.tile_pool(name="ps_mm", bufs=3, space="PSUM"))
        ps_tiny = ctx.enter_context(tc.tile_pool(name="ps_tiny", bufs=2, space="PSUM"))
        dram = ctx.enter_context(tc.tile_pool(name="dram", bufs=1, space="DRAM"))

        def cload(pool, pname, shape, dt, rearr=None):
            t = pool.tile(list(shape), dt, name=pname + "_s")
            src = P[pname][:]
            if rearr is not None:
                src = src.rearrange(rearr[0], **rearr[1])
            nc.sync.dma_start(out=t[:], in_=src)
            return t

        wqab_s = cload(cst, "wqab", (128, 8, KH + 2), BF16,
                       ("(kc p) f -> p kc f", dict(p=128)))
        wk_s = cload(cst, "wk", (128, 4, KH), BF16,
                     ("(kc p) f -> p kc f", dict(p=128)))
        wv_s = cload(cst, "wv", (128, 4, VH), BF16,
                     ("(kc p) f -> p kc f", dict(p=128)))
        cqd_s = cload(cst, "cqd", (KH, KS, KH), BF16, ("s p f -> p s f", {}))
        ckd_s = cload(cst, "ckd", (KH, KS, KH), BF16, ("s p f -> p s f", {}))
        cvd_s = cload(cst, "cvd", (KH, KS, 2, KH), BF16, ("s h p f -> p s h f", {}))
        scal_s = cload(cst, "scal", (1, 8), F32)
        png_s = cload(cst, "png", (128, 12), F32,
                      ("(ct p) o -> p (ct o)", dict(p=128)))
        maskS_s = cload(cst, "maskS", (C, C), F32)
        maskI_s = cload(cst, "maskI", (C, C), F32)
        id128f_s = cload(cst, "id128f", (128, 128), F32)
        id128b_s = cload(cst, "id128b", (128, 128), BF16)
        id96b_s = cload(cst, "id96b", (96, 96), BF16)
        id1b_s = cload(cst, "id1b", (1, 1), BF16)
        onesb_s = cload(cst, "onesb", (1, 128), BF16)
        monesb_s = cload(cst, "monesb", (1, 128), BF16)
        one1b_s = cload(cst, "one1b", (1, 1), BF16)
        selq_s = cload(cst, "selq", (B * NCHUNK, 4), F32)
        ones2b_s = cload(cst, "ones2b", (2, 128), BF16)
        mones2b_s = cload(cst, "mones2b", (2, 128), BF16)
        one2b_s = cload(cst, "one2b", (2, 1), BF16)
        id4f_s = cload(cst, "id4f", (4, 4), F32)
        ones96b_s = cst.tile([KH, 1], BF16)
        nc.vector.memset(ones96b_s[:], 1.0)
        epsb_s = cst.tile([C, 1], F32)
        nc.vector.memset(epsb_s[:], EPS)
        one1f_s = cst.tile([1, 1], F32)
        nc.vector.memset(one1f_s[:], 1.0)

        # persistent outputs of the scan
        ssqc = pers.tile([C, B * NCHUNK], F32)
        gate = [pers.tile([128, TOKSLC], BF16, name=f"gate{ct}") for ct in range(12)]

        # ---------------- A) projections ----------------
        work_cm = tc.tile_pool(name="work", bufs=1)
        work = work_cm.__enter__()
        rawp_cm = tc.tile_pool(name="rawp", bufs=1)
        rawp = rawp_cm.__enter__()
        qraw = [rawp.tile([KH, 3 + T], BF16, name=f"qraw{p}") for p in range(B)]
        kraw = [rawp.tile([KH, 3 + T], BF16, name=f"kraw{p}") for p in range(B)]
        v0raw = [rawp.tile([KH, 3 + T], BF16, name=f"v0raw{p}") for p in range(B)]
        v1raw = [rawp.tile([KH, 3 + T], BF16, name=f"v1raw{p}") for p in range(B)]
        abd = [dram.tile([2, T], F32, name=f"abd{p}") for p in range(B)]
        for p in range(B):
            for t_ in (qraw[p], kraw[p], v0raw[p], v1raw[p]):
                nc.vector.memset(t_[:, 0:3], 0.0)

        for tt in range(8):
            p, lt = tt // 4, tt % 4
            ps_q = ps_acc.tile([KH + 2, 512], F32, tag="acc")
            for kc in range(8):
                xt = strm.tile([128, 512], BF16, tag="xt")
                nc.gpsimd.dma_start(
                    out=xt[:], in_=xT[kc * 128:(kc + 1) * 128,
                                      tt * 512:(tt + 1) * 512])
                nc.tensor.matmul(ps_q[:], lhsT=wqab_s[:, kc, :],
                                 rhs=xt[:], start=(kc == 0), stop=(kc == 7))
            nc.vector.tensor_copy(out=qraw[p][:, 3 + lt * 512: 3 + (lt + 1) * 512],
                           in_=ps_q[0:KH, :])
            abev = scr.tile([2, 512], F32, tag="abev", bufs=1)
            nc.vector.tensor_copy(out=abev[:], in_=ps_q[KH:KH + 2, :])
            nc.sync.dma_start(out=abd[p][:, lt * 512:(lt + 1) * 512], in_=abev[:])

            ps_k = ps_acc.tile([KH, 512], F32, tag="acc")
            ps_v0 = ps_mm.tile([KH, 512], F32, tag="mm")
            ps_v1 = ps_mm.tile([KH, 512], F32, tag="mm")
            for kc in range(4):
                ct = strm.tile([128, 512], BF16, tag="ct")
                nc.gpsimd.dma_start(
                    out=ct[:], in_=ckvT[kc * 128:(kc + 1) * 128,
                                        tt * 512:(tt + 1) * 512])
                nc.tensor.matmul(ps_k[:], lhsT=wk_s[:, kc, :],
                                 rhs=ct[:], start=(kc == 0), stop=(kc == 3))
                nc.tensor.matmul(ps_v0[:], lhsT=wv_s[:, kc, 0:KH],
                                 rhs=ct[:], start=(kc == 0), stop=(kc == 3))
                nc.tensor.matmul(ps_v1[:], lhsT=wv_s[:, kc, KH:VH],
                                 rhs=ct[:], start=(kc == 0), stop=(kc == 3))
            nc.vector.tensor_copy(out=kraw[p][:, 3 + lt * 512: 3 + (lt + 1) * 512],
                           in_=ps_k[:])
            nc.vector.tensor_copy(out=v0raw[p][:, 3 + lt * 512: 3 + (lt + 1) * 512],
                           in_=ps_v0[:])
            nc.vector.tensor_copy(out=v1raw[p][:, 3 + lt * 512: 3 + (lt + 1) * 512],
                           in_=ps_v1[:])

        # ---------------- B) conv + SiLU (bf16 outs) ----------------
        qn = [work.tile([KH, T], BF16, name=f"qn{p}") for p in range(B)]
        kn = [work.tile([KH, T], BF16, name=f"kn{p}") for p in range(B)]
        vc0 = [work.tile([KH, T], BF16, name=f"vc0{p}") for p in range(B)]
        vc1 = [work.tile([KH, T], BF16, name=f"vc1{p}") for p in range(B)]
        conv_jobs = []
        for p in range(B):
            conv_jobs += [(qraw[p], cqd_s, qn[p], None),
                          (kraw[p], ckd_s, kn[p], None),
                          (v0raw[p], cvd_s, vc0[p], 0),
                          (v1raw[p], cvd_s, vc1[p], 1)]
        for raw, dg, dst, vh in conv_jobs:
            for lt in range(4):
                ps_c = ps_mm.tile([KH, 512], F32, tag="mm")
                for s in range(KS):
                    lhs = dg[:, s, vh, :] if vh is not None else dg[:, s, :]
                    nc.tensor.matmul(ps_c[:], lhsT=lhs,
                                     rhs=raw[:, lt * 512 + s: lt * 512 + s + 512],
                                     start=(s == 0), stop=(s == KS - 1))
                nc.scalar.activation(out=dst[:, lt * 512:(lt + 1) * 512],
                                     in_=ps_c[:], func=AF.Silu)
        rawp_cm.__exit__(None, None, None)

        # gate weights reuse the raw tiles' space (gate GEMM overlaps the scan)
        mid_cm = tc.tile_pool(name="mid", bufs=1)
        mid = mid_cm.__enter__()
        gw_s = [mid.tile([128, VT], BF16, name=f"gwt{kc}") for kc in range(8)]
        xs_s = [mid.tile([128, TOKSLC], BF16, name=f"xst{kc}") for kc in range(8)]
        for kcb in range(8):
            nc.gpsimd.dma_start(out=gw_s[kcb][:],
                                in_=gw[kcb * 128:(kcb + 1) * 128, :])
            nc.gpsimd.dma_start(out=xs_s[kcb][:],
                                in_=xsT[kcb * 128:(kcb + 1) * 128, :])

        # ---------------- C) L2 norm of q,k (in place) ----------------
        for p in range(B):
            for src, qscale in ((qn[p], KH ** -0.5), (kn[p], None)):
                for lt in range(4):
                    sl = slice(lt * 512, (lt + 1) * 512)
                    sq = scr.tile([KH, 512], BF16, tag="sq")
                    nc.vector.tensor_tensor(out=sq[:], in0=src[:, sl],
                                            in1=src[:, sl], op=AL.mult)
                    ps_n = ps_tiny.tile([1, 512], F32, tag="tiny")
                    nc.tensor.matmul(ps_n[:], lhsT=ones96b_s[:], rhs=sq[:],
                                     start=True, stop=True)
                    # factor = qscale/sqrt(ssq) = rsqrt(ssq/qscale^2): one ACT
                    # Rsqrt (eps negligible: ||q|| >> 1e-6 for silu-conv outs).
                    nrb = scr.tile([1, 512], BF16, tag="recb")
                    iscale = (1.0 / float(qscale) ** 2 if qscale is not None
                              else 1.0)
                    eng = nc.scalar
                    eng.add_instruction(mybir.InstActivation(
                        name=nc.get_next_instruction_name(),
                        func=AF.Rsqrt,
                        ins=[eng.lower_ap(ps_n[:]),
                             mybir.ImmediateValue(dtype=F32, value=0.0),
                             mybir.ImmediateValue(dtype=F32, value=iscale),
                             mybir.ImmediateValue(dtype=F32, value=0.0)],
                        outs=[eng.lower_ap(nrb[:])]))
                    ps_f = ps_mm.tile([KH, 512], F32, tag="mm")
                    nc.tensor.matmul(ps_f[:], lhsT=onesb_s[0:1, 0:KH],
                                     rhs=nrb[:], start=True, stop=True)
                    nc.vector.tensor_tensor(out=src[:, sl], in0=src[:, sl],
                                            in1=ps_f[:], op=AL.mult)

        # ---------------- D) decay rows + cumsum (bf16 hi/lo rows) -----------
        Lhl = [work.tile([2, T], BF16, name=f"Lhl{p}") for p in range(B)]
        Lh = [work.tile([1, T], BF16, name=f"Lh{p}") for p in range(B)]
        Ll = [work.tile([1, T], BF16, name=f"Ll{p}") for p in range(B)]
        brow = [work.tile([1, T], BF16, name=f"brow{p}") for p in range(B)]
        lnbrow = [work.tile([1, T], BF16, name=f"lnbrow{p}") for p in range(B)]
        for p in range(B):
            arow = scr.tile([1, T], F32, tag="rowC", bufs=1)
            nc.sync.dma_start(out=arow[:], in_=abd[p][0:1, :])
            rsc0 = scr.tile([1, T], F32, tag="rowD", bufs=1)
            nc.scalar.activation(out=rsc0[:], in_=arow[:], func=AF.Exp,
                                 bias=scal_s[0:1, 0:1], scale=1.0)
            rsc = scr.tile([1, T], F32, tag="rowC", bufs=1)
            nc.scalar.activation(out=rsc[:], in_=rsc0[:], func=AF.Ln,
                                 bias=one1f_s[:], scale=1.0)
            brawrow = scr.tile([1, T], F32, tag="rowC", bufs=1)
            nc.sync.dma_start(out=brawrow[:], in_=abd[p][1:2, :])
            grow = scr.tile([1, T], F32, tag="rowB", bufs=1)
            nc.vector.tensor_scalar(out=grow[:], in0=rsc[:],
                                    scalar1=scal_s[0:1, 1:2], scalar2=None,
                                    op0=AL.mult)
            nc.scalar.activation(out=brow[p][:], in_=brawrow[:],
                                 func=AF.Sigmoid, bias=scal_s[0:1, 2:3], scale=1.0)
            lnb_e = scr.tile([1, T], F32, tag="rowD", bufs=1)
            nc.scalar.activation(out=lnb_e[:], in_=brawrow[:], func=AF.Exp,
                                 bias=scal_s[0:1, 3:4], scale=-1.0)
            lnb_t = scr.tile([1, T], F32, tag="rowC", bufs=1)
            nc.scalar.activation(out=lnb_t[:], in_=lnb_e[:], func=AF.Ln,
                                 bias=one1f_s[:], scale=1.0)
            nc.vector.tensor_scalar_mul(out=lnbrow[p][:], in0=lnb_t[:], scalar1=-1.0)
            gb = dram.tile([NCHUNK, C], F32, name=f"gb{p}")
            nc.sync.dma_start(
                out=gb[:].rearrange("p f -> (p f)").unsqueeze(0), in_=grow[:])
            g16 = scr.tile([NCHUNK, C], F32, tag="g16")
            nc.sync.dma_start(out=g16[:], in_=gb[:])
            L16t = scr.tile([NCHUNK, C], F32, tag="L16t")
            nc.vector.tensor_tensor_scan(out=L16t[:], data0=g16[:], data1=g16[:],
                                         initial=0.0, op0=AL.add, op1=AL.bypass)
            gb2 = dram.tile([NCHUNK, C], F32, name=f"gb2{p}")
            nc.sync.dma_start(out=gb2[:], in_=L16t[:])
            Lrow = scr.tile([1, T], F32, tag="rowC", bufs=1)
            nc.sync.dma_start(
                out=Lrow[:], in_=gb2[:].rearrange("p f -> (p f)").unsqueeze(0))
            nc.vector.tensor_copy(out=Lh[p][:], in_=Lrow[:])
            nc.vector.tensor_tensor(out=Ll[p][:], in0=Lrow[:], in1=Lh[p][:],
                                    op=AL.subtract)
            lbd = dram.tile([2, T], BF16, name=f"lbd{p}")
            nc.sync.dma_start(out=lbd[0:1, :], in_=Lh[p][:])
            nc.sync.dma_start(out=lbd[1:2, :], in_=Ll[p][:])
            nc.sync.dma_start(out=Lhl[p][:], in_=lbd[:])

        # -------- E/F/G) per-chunk prep + sequential sweep + output ----------
        a2ain = [dram.tile([8, VH, TOKSLC // 2], BF16, name=f"a2ain{h_}")
                 for h_ in range(2)]
        a2aout = [dram.tile([8, VH, TOKSLC // 2], BF16, name=f"a2aout{h_}")
                  for h_ in range(2)]
        Scur = []
        for p in range(B):
            s0 = ring.tile([KH, VH], BF16, tag=f"Sh{p}", bufs=4, name=f"S0_{p}")
            nc.vector.memset(s0[:], 0.0)
            Scur.append(s0)

        for i in range(NCHUNK):
            for p in range(B):
                j = i * B + p
                ck = slice(i * C, (i + 1) * C)
                lstc = slice((i + 1) * C - 1, (i + 1) * C)
                Lrh, Lrl = Lh[p][0:1, ck], Ll[p][0:1, ck]

                # E[t,s] = L_t - L_s via 4 rank-1 bf16 matmuls (hi/lo
                # exact); E, Gkk, Gqk packed into one psum bank.
                ps_egg = ps_mm.tile([C, 3, C], F32, tag="mm")
                ps_e = ps_egg[:, 0]
                nc.tensor.matmul(ps_e, lhsT=Lhl[p][:, ck], rhs=ones2b_s[:],
                                 start=True, stop=False)
                nc.tensor.matmul(ps_e, lhsT=mones2b_s[:], rhs=Lhl[p][:, ck],
                                 start=False, stop=True)
                Es = scr.tile([C, C], F32, tag="Es", bufs=3)
                nc.vector.tensor_tensor(out=Es[:], in0=ps_e, in1=maskS_s[:],
                                        op=AL.add)
                Ei = scr.tile([C, C], F32, tag="Ei", bufs=3)
                nc.vector.tensor_tensor(out=Ei[:], in0=ps_e, in1=maskI_s[:],
                                        op=AL.add)

                # lnb / beta columns
                ps_lb = ps_tiny.tile([C, 4], BF16, tag="tiny")
                nc.tensor.transpose(ps_lb[:, 0:1], lnbrow[p][0:1, ck], id1b_s[:])
                nc.tensor.transpose(ps_lb[:, 2:3], brow[p][0:1, ck], id1b_s[:])
                lnbc = scr.tile([C, 1], F32, tag="lnbc")
                nc.scalar.copy(out=lnbc[:], in_=ps_lb[:, 0:1])
                bc = scr.tile([C, 1], F32, tag="bc")
                nc.scalar.copy(out=bc[:], in_=ps_lb[:, 2:3])

                Mexp = scr.tile([C, C], F32, tag="Mexp", bufs=3)
                nc.scalar.activation(out=Mexp[:], in_=Es[:], func=AF.Exp,
                                     bias=lnbc[:], scale=1.0)
                Dincl = scr.tile([C, C], F32, tag="Dincl", bufs=3)
                nc.scalar.activation(out=Dincl[:], in_=Ei[:], func=AF.Exp)

                # N (strict-lower, includes beta) and its transpose
                ps_gk = ps_egg[:, 1]
                nc.tensor.matmul(ps_gk, lhsT=kn[p][:, ck], rhs=kn[p][:, ck],
                                 start=True, stop=True)
                Nbf = scr.tile([C, C], BF16, tag="Nbf", bufs=3)
                nc.vector.scalar_tensor_tensor(out=Nbf[:], in0=ps_gk,
                                               scalar=-1.0, in1=Mexp[:],
                                               op0=AL.mult, op1=AL.mult)
                # P = (q_t . k_s) * Dincl
                ps_gq = ps_egg[:, 2]
                nc.tensor.matmul(ps_gq, lhsT=qn[p][:, ck], rhs=kn[p][:, ck],
                                 start=True, stop=True)
                Pbf = scr.tile([C, C], BF16, tag="Pbf", bufs=3)
                nc.vector.tensor_tensor(out=Pbf[:], in0=ps_gq, in1=Dincl[:],
                                        op=AL.mult)
                # both transposes (N, P) share one psum bank
                ps_nt = ps_mm.tile([C, 2, C], BF16, tag="mm")
                nc.tensor.transpose(ps_nt[:, 0], Nbf[:], id128b_s[:])
                nc.tensor.transpose(ps_nt[:, 1], Pbf[:], id128b_s[:])
                ev = nc.scalar.copy if (j % 2 == 0) else \
                    (lambda out, in_: nc.vector.tensor_copy(out=out, in_=in_))
                NTbf = scr.tile([C, C], BF16, tag="NTbf", bufs=3)
                ev(out=NTbf[:], in_=ps_nt[:, 0])
                P0b = scr.tile([C, C], BF16, tag="P0b", bufs=3)
                nc.vector.tensor_tensor(out=P0b[:], in0=ps_nt[:, 0],
                                        in1=id128f_s[:], op=AL.add)
                PTb = ring.tile([C, C], BF16, tag="PTb", name=f"PTb{j}")
                nc.scalar.copy(out=PTb[:], in_=ps_nt[:, 1])

                # doubling powers (levels=3)
                ps_sq = ps_mm.tile([C, 2, C], F32, tag="mm")
                nc.tensor.matmul(ps_sq[:, 0], lhsT=NTbf[:], rhs=Nbf[:],
                                 start=True, stop=True)
                nc.tensor.matmul(ps_sq[:, 1], lhsT=Nbf[:], rhs=NTbf[:],
                                 start=True, stop=True)
                N2r = scr.tile([C, C], BF16, tag="N2r", bufs=3)
                ev(out=N2r[:], in_=ps_sq[:, 0])
                N2i = scr.tile([C, C], BF16, tag="N2i", bufs=3)
                nc.vector.tensor_tensor(out=N2i[:], in0=ps_sq[:, 0],
                                        in1=id128f_s[:], op=AL.add)
                N2Tr = scr.tile([C, C], BF16, tag="N2Tr", bufs=3)
                ev(out=N2Tr[:], in_=ps_sq[:, 1])
                ps_sq3 = ps_mm.tile([C, 2, C], F32, tag="mm")
                nc.tensor.matmul(ps_sq3[:, 0], lhsT=N2Tr[:], rhs=N2r[:],
                                 start=True, stop=True)
                nc.tensor.matmul(ps_sq3[:, 1], lhsT=N2r[:], rhs=N2Tr[:],
                                 start=True, stop=True)
                N4r = scr.tile([C, C], BF16, tag="N4r", bufs=3)
                ev(out=N4r[:], in_=ps_sq3[:, 0])
                N4i = scr.tile([C, C], BF16, tag="N4i", bufs=3)
                nc.vector.tensor_tensor(out=N4i[:], in0=ps_sq3[:, 0],
                                        in1=id128f_s[:], op=AL.add)
                N4Tr = scr.tile([C, C], BF16, tag="N4Tr", bufs=3)
                ev(out=N4Tr[:], in_=ps_sq3[:, 1])
                ps_sq5 = ps_mm.tile([C, C], F32, tag="mm")
                nc.tensor.matmul(ps_sq5[:], lhsT=N4Tr[:], rhs=N4r[:],
                                 start=True, stop=True)
                N8i = scr.tile([C, C], BF16, tag="N8i", bufs=3)
                nc.vector.tensor_tensor(out=N8i[:], in0=ps_sq5[:], in1=id128f_s[:],
                                        op=AL.add)

                # chain: T^T = (I+N8T)(I+N4T)(I+N2T)(I+NT)
                ps_c1 = ps_mm.tile([C, C], F32, tag="mm")
                nc.tensor.matmul(ps_c1[:], lhsT=N2i[:], rhs=P0b[:],
                                 start=True, stop=True)
                C1 = scr.tile([C, C], BF16, tag="C1", bufs=3)
                ev(out=C1[:], in_=ps_c1[:])
                ps_c2 = ps_mm.tile([C, C], F32, tag="mm")
                nc.tensor.matmul(ps_c2[:], lhsT=N4i[:], rhs=C1[:],
                                 start=True, stop=True)
                C2 = scr.tile([C, C], BF16, tag="C2", bufs=3)
                ev(out=C2[:], in_=ps_c2[:])
                ps_c3 = ps_mm.tile([C, C], F32, tag="mm")
                nc.tensor.matmul(ps_c3[:], lhsT=N8i[:], rhs=C2[:],
                                 start=True, stop=True)
                TTm = ring.tile([C, C], BF16, tag="TTm", name=f"TTm{j}")
                nc.scalar.copy(out=TTm[:], in_=ps_c3[:])

                # V halves + K token-layout transposes share one psum bank
                bV = ring.tile([C, VH], BF16, tag="bV", name=f"bV{j}")
                ps_vt = ps_mm.tile([C, 3, KH], BF16, tag="mm")
                nc.tensor.transpose(ps_vt[:, 0], vc0[p][:, ck], id96b_s[:])
                nc.tensor.transpose(ps_vt[:, 1], vc1[p][:, ck], id96b_s[:])
                nc.tensor.transpose(ps_vt[:, 2], kn[p][:, ck], id96b_s[:])
                for hh in range(2):
                    nc.vector.tensor_scalar(out=bV[:, hh * KH:(hh + 1) * KH],
                                            in0=ps_vt[:, hh], scalar1=bc[:],
                                            scalar2=None, op0=AL.mult)
                ps_kt = ps_vt[:, 2]
                ps_ll3 = ps_tiny.tile([C, 3], F32, tag="tiny")
                nc.tensor.matmul(ps_ll3[:, 0:1], lhsT=ones2b_s[:],
                                 rhs=Lhl[p][:, lstc], start=True, stop=True)
                Llc = scr.tile([C, 1], F32, tag="Llc")
                nc.scalar.copy(out=Llc[:], in_=ps_ll3[:, 0:1])
                nc.tensor.matmul(ps_ll3[:, 1:2], lhsT=Lhl[p][:, ck],
                                 rhs=one2b_s[:], start=True, stop=True)
                eLl = scr.tile([C, 1], F32, tag="eLl")
                nc.scalar.activation(out=eLl[:], in_=ps_ll3[:, 1:2], func=AF.Exp,
                                     bias=Llc[:], scale=-1.0)
                Ktok = ring.tile([C, KH], BF16, tag="Ktok", name=f"Ktok{j}")
                nc.vector.tensor_scalar(out=Ktok[:], in0=ps_kt, scalar1=eLl[:],
                                        scalar2=None, op0=AL.mult)

                # A_t column, beta*A column, chunk-total decay
                Acol = ring.tile([C, 1], F32, tag="Acol", name=f"Acol{j}")
                nc.scalar.activation(out=Acol[:], in_=ps_ll3[:, 1:2], func=AF.Exp)
                bAcol = ring.tile([C, 1], F32, tag="bAcol", name=f"bAcol{j}")
                nc.vector.tensor_tensor(out=bAcol[:], in0=Acol[:], in1=bc[:],
                                        op=AL.mult)
                nc.tensor.matmul(ps_ll3[0:KH, 2:3], lhsT=ones2b_s[:, 0:KH],
                                 rhs=Lhl[p][:, lstc], start=True, stop=True)
                aC96 = ring.tile([KH, 1], F32, tag="aC96", name=f"aC96{j}")
                nc.scalar.activation(out=aC96[:], in_=ps_ll3[0:KH, 2:3],
                                     func=AF.Exp)

                # ---- sequential sweep step ----
                ps_y = ps_mm.tile([C, VH], F32, tag="mm")
                nc.tensor.matmul(ps_y[:], lhsT=kn[p][:, ck], rhs=Scur[p][:],
                                 start=True, stop=True)
                R2 = scr.tile([C, VH], BF16, tag="R2")
                nc.vector.scalar_tensor_tensor(out=R2[:], in0=ps_y[:],
                                               scalar=bAcol[:], in1=bV[:],
                                               op0=AL.mult, op1=AL.subtract)
                ps_u = ps_mm.tile([C, VH], F32, tag="mm")
                nc.tensor.matmul(ps_u[:], lhsT=TTm[:], rhs=R2[:],
                                 start=True, stop=True)
                U = ring.tile([C, VH], BF16, tag="U", name=f"U{j}")
                nc.scalar.activation(out=U[:], in_=ps_u[:], func=AF.Copy,
                                     scale=-1.0)
                ps_s = ps_mm.tile([KH, VH], F32, tag="mm")
                nc.tensor.matmul(ps_s[:], lhsT=Ktok[:], rhs=U[:],
                                 start=True, stop=True)
                Snew = ring.tile([KH, VH], BF16, tag=f"Sh{p}", bufs=4,
                                 name=f"S{p}_{i + 1}")
                nc.vector.scalar_tensor_tensor(out=Snew[:], in0=Scur[p][:],
                                               scalar=aC96[:], in1=ps_s[:],
                                               op0=AL.mult, op1=AL.add)

                # ---- output epilogue ----
                ps_pu = ps_acc.tile([C, VH], F32, tag="acc")
                nc.tensor.matmul(ps_pu[:], lhsT=PTb[:], rhs=U[:],
                                 start=True, stop=True)
                ps_z = ps_acc.tile([C, VH], F32, tag="acc")
                nc.tensor.matmul(ps_z[:], lhsT=qn[p][:, ck], rhs=Scur[p][:],
                                 start=True, stop=True)
                tmpZ = scr.tile([C, VH], F32, tag="tmpZ")
                nc.vector.tensor_scalar(out=tmpZ[:], in0=ps_z[:], scalar1=Acol[:],
                                        scalar2=None, op0=AL.mult)
                Osb = scr.tile([C, VH], BF16, tag="Osb", bufs=3)
                nc.vector.tensor_tensor(out=Osb[:], in0=ps_pu[:], in1=tmpZ[:],
                                        op=AL.add)
                sqo = scr.tile([C, VH], BF16, tag="sqo")
                nc.vector.scalar_tensor_tensor(out=sqo[:], in0=Osb[:],
                                               scalar=1.0, in1=Osb[:],
                                               op0=AL.mult, op1=AL.mult,
                                               accum_out=ssqc[:, j:j + 1])
                # transpose + ship (unnormalized) o to the A2A send buffer;
                # the RMS factor is applied per-token at the final GEMM evict.
                tok0 = p * T + i * C
                d, off = tok0 // TOKSLC, tok0 % TOKSLC
                half, off2 = (0, off) if off < TOKSLC // 2 else (1, off - TOKSLC // 2)
                ps_ot = ps_mm.tile([KH, 2, C], BF16, tag="mm")
                nc.tensor.transpose(ps_ot[:, 0], Osb[:, 0:KH], id128b_s[:])
                nc.tensor.transpose(ps_ot[:, 1], Osb[:, KH:VH], id128b_s[:])
                for hh in range(2):
                    otb = scr.tile([KH, C], BF16, tag="otb")
                    ev(out=otb[:], in_=ps_ot[:, hh])
                    nc.sync.dma_start(
                        out=a2ain[half][d, hh * KH:(hh + 1) * KH, off2:off2 + C],
                        in_=otb[:])
                Scur[p] = Snew
                if i == NCHUNK - 3 and p == B - 1:
                    # all chunks of the first token-half are now emitted
                    nc.gpsimd.collective_compute(
                        "AllToAll", AL.bypass, replica_groups=[list(range(8))],
                        ins=[a2ain[0].opt()], outs=[a2aout[0].opt()])

        # ---------------- K) gate GEMM (overlaps scan via dataflow) ----------
        for ctb in range(12):
            ps_g = ps_gate.tile([128, TOKSLC], F32, tag="gate")
            for kcb in range(8):
                nc.tensor.matmul(ps_g[:],
                                 lhsT=gw_s[kcb][:, ctb * 128:(ctb + 1) * 128],
                                 rhs=xs_s[kcb][:], start=(kcb == 0), stop=(kcb == 7))
            nc.scalar.activation(out=gate[ctb][:], in_=ps_g[:], func=AF.Silu)

        # ---------------- H) ssq AllReduce + rsqrt ----------------
        arin = dram.tile([C, B * NCHUNK], F32, name="arin")
        arout = dram.tile([C, B * NCHUNK], F32, name="arout")
        nc.sync.dma_start(out=arin[:], in_=ssqc[:])
        nc.gpsimd.collective_compute(
            "AllReduce", AL.add, replica_groups=[list(range(8))],
            ins=[arin.opt()], outs=[arout.opt()])
        rq = pers.tile([C, B * NCHUNK], F32)
        nc.sync.dma_start(out=rq[:], in_=arout[:])
        rb = pers.tile([C, B * NCHUNK], F32)
        nc.scalar.activation(out=rb[:], in_=rq[:], func=AF.Sqrt,
                             bias=epsb_s[:], scale=1.0 / VT)
        rs = pers.tile([C, B * NCHUNK], F32)
        nc.vector.reciprocal(out=rs[:], in_=rb[:])
        # gather this core's 4 per-token-block rs columns via one-hot matmul
        ps_rt = ps_tiny.tile([B * NCHUNK, C], F32, tag="tiny")
        nc.tensor.transpose(ps_rt[:], rs[:], id128f_s[:])
        rsT = pers.tile([B * NCHUNK, C], F32)
        nc.scalar.copy(out=rsT[:], in_=ps_rt[:])
        ps_r4 = ps_tiny.tile([4, C], F32, tag="tiny")
        nc.tensor.matmul(ps_r4[:], lhsT=selq_s[:], rhs=rsT[:],
                         start=True, stop=True)
        rs4T = pers.tile([4, C], F32)
        nc.scalar.copy(out=rs4T[:], in_=ps_r4[:])
        ps_rq = ps_tiny.tile([C, 4], F32, tag="tiny")
        nc.tensor.transpose(ps_rq[:], rs4T[:], id4f_s[:])
        rsq_sb = pers.tile([C, 4], F32)
        nc.scalar.copy(out=rsq_sb[:], in_=ps_rq[:])
        rsq = [rsq_sb[:, to:to + 1] for to in range(4)]

        # second-half A2A (first half fires inside the chunk loop)
        nc.gpsimd.collective_compute(
            "AllToAll", AL.bypass, replica_groups=[list(range(8))],
            ins=[a2ain[1].opt()], outs=[a2aout[1].opt()])

        mid_cm.__exit__(None, None, None)
        work_cm.__exit__(None, None, None)

        # ---------------- L) OG product + final GEMM ----------------
        late_cm = tc.tile_pool(name="late", bufs=1)
        late = late_cm.__enter__()
        wo_s = [late.tile([128, DM], BF16, name=f"wot{ct}") for ct in range(12)]
        for ct in range(12):
            nc.gpsimd.dma_start(out=wo_s[ct][:],
                                in_=wo[ct * 128:(ct + 1) * 128, :])
        og = [late.tile([128, TOKSLC], BF16, name=f"og{ct}") for ct in range(12)]
        HT = TOKSLC // 2
        flats = [a2aout[h_][:].rearrange("h c t -> (h c) t") for h_ in range(2)]
        for half in range(2):
            for ct in range(12):
                ogin = late.tile([128, HT], BF16, tag="ogin", bufs=3,
                                 name=f"ogin{half}_{ct}")
                nc.gpsimd.dma_start(out=ogin[:],
                                    in_=flats[half][ct * 128:(ct + 1) * 128, :])
                nc.vector.scalar_tensor_tensor(
                    out=og[ct][:, half * HT:(half + 1) * HT], in0=ogin[:],
                    scalar=png_s[:, ct:ct + 1],
                    in1=gate[ct][:, half * HT:(half + 1) * HT],
                    op0=AL.mult, op1=AL.mult)
        for to in range(4):
            # rs column for this 128-token block of this core's slice
            # (core id is data-independent: token block -> (p, i) -> ssqc col)
            for fo in range(2):
                ps_o = ps_gate.tile([128, 512], F32, tag="gate")
                for ct in range(12):
                    nc.tensor.matmul(ps_o[:],
                                     lhsT=og[ct][:, to * 128:(to + 1) * 128],
                                     rhs=wo_s[ct][:, fo * 512:(fo + 1) * 512],
                                     start=(ct == 0), stop=(ct == 11))
                osb = late.tile([128, 512], F32, tag="osb", bufs=2,
                                name=f"osb{to}_{fo}")
                nc.vector.tensor_scalar(out=osb[:], in0=ps_o[:],
                                        scalar1=rsq[to][:], scalar2=None,
                                        op0=AL.mult)
                nc.sync.dma_start(
                    out=out[to * 128:(to + 1) * 128, fo * 512:(fo + 1) * 512],
                    in_=osb[:])
        late_cm.__exit__(None, None, None)
        ctx.close()

    return nc


def kernel(x, c_kv, w_q, w_k, w_v, conv_q_w, conv_q_b, conv_k_w, conv_k_b,
           conv_v_w, conv_v_b, a_proj_w, a_proj_b, A_log, dt_bias,
           b_proj_w, b_proj_b, g_proj_w, post_norm_w, w_o):
    from concourse.bass_utils import run_bass_kernel_spmd

    bf = ml_dtypes.bfloat16
    x = np.asarray(x, np.float32)
    c_kv = np.asarray(c_kv, np.float32)
    xT = np.ascontiguousarray(x.reshape(NTOK, DM).T).astype(bf)
    ckvT = np.ascontiguousarray(c_kv.reshape(NTOK, DKV).T).astype(bf)
    gw = np.asarray(g_proj_w, np.float32).astype(bf)
    wo_ = np.asarray(w_o, np.float32).astype(bf)
    png = np.asarray(post_norm_w, np.float32).reshape(VT, 1)

    maskS = np.where(np.arange(C)[None, :] < np.arange(C)[:, None], 0.0, NEG)
    maskI = np.where(np.arange(C)[None, :] <= np.arange(C)[:, None], 0.0, NEG)
    consts = dict(
        maskS=maskS.astype(np.float32), maskI=maskI.astype(np.float32),
        id128f=np.eye(128, dtype=np.float32),
        id128b=np.eye(128, dtype=np.float32).astype(bf),
        id96b=np.eye(96, dtype=np.float32).astype(bf),
        id1b=np.ones((1, 1), np.float32).astype(bf),
        onesb=np.ones((1, 128), np.float32).astype(bf),
        monesb=(-np.ones((1, 128), np.float32)).astype(bf),
        one1b=np.ones((1, 1), np.float32).astype(bf),
        id4f=np.eye(4, dtype=np.float32),
        ones2b=np.ones((2, 128), np.float32).astype(bf),
        mones2b=(-np.ones((2, 128), np.float32)).astype(bf),
        one2b=np.ones((2, 1), np.float32).astype(bf),
    )

    in_maps = []
    for c in range(8):
        h = c
        qs = slice(h * KH, (h + 1) * KH)
        vs = slice(h * VH, (h + 1) * VH)
        wqab_ = np.concatenate([
            np.asarray(w_q, np.float32)[:, qs],
            np.asarray(a_proj_w, np.float32)[:, h:h + 1],
            np.asarray(b_proj_w, np.float32)[:, h:h + 1]], axis=1).astype(bf)
        cq = np.asarray(conv_q_w, np.float32)[qs, 0, :]
        ck = np.asarray(conv_k_w, np.float32)[qs, 0, :]
        cv = np.asarray(conv_v_w, np.float32)[vs, 0, :]
        cqd_ = np.stack([np.diag(cq[:, s]) for s in range(KS)]).astype(bf)
        ckd_ = np.stack([np.diag(ck[:, s]) for s in range(KS)]).astype(bf)
        cvd_ = np.stack([np.stack([np.diag(cv[hh * KH:(hh + 1) * KH, s])
                                   for hh in range(2)])
                         for s in range(KS)]).astype(bf)
        scal_ = np.zeros((1, 8), np.float32)
        scal_[0, 0] = float(np.asarray(dt_bias)[h] + np.asarray(a_proj_b)[h])
        scal_[0, 1] = -float(np.exp(np.asarray(A_log)[h]))
        scal_[0, 2] = float(np.asarray(b_proj_b)[h])
        scal_[0, 3] = -float(np.asarray(b_proj_b)[h])
        selq = np.zeros((B * NCHUNK, 4), np.float32)
        for to in range(4):
            tok0 = c * TOKSLC + to * 128
            p_, i_ = tok0 // T, (tok0 % T) // C
            selq[i_ * B + p_, to] = 1.0
        m = dict(
            selq=selq,
            xT=xT, ckvT=ckvT, wqab=wqab_,
            wk=np.asarray(w_k, np.float32)[:, qs].astype(bf),
            wv=np.asarray(w_v, np.float32)[:, vs].astype(bf),
            cqd=cqd_, ckd=ckd_, cvd=cvd_, scal=scal_, gw=gw, wo=wo_,
            xsT=np.ascontiguousarray(xT[:, c * TOKSLC:(c + 1) * TOKSLC]),
            png=png, **consts)
        in_maps.append(m)

    if "nc" not in _CACHE:
        _CACHE["nc"] = _build()
    res = run_bass_kernel_spmd(_CACHE["nc"], in_maps, core_ids=list(range(8)))
    _CACHE["last"] = res
    parts = [np.asarray(res.results[c]["out"], np.float32) for c in range(8)]
    return np.concatenate(parts, axis=0).reshape(B, T, DM)


# revision 28
# speedup vs baseline: 1.2126x; 1.1886x over previous
"""nn_GatedDeltaRecurrence Trainium2 kernel (8 NeuronCores, Bass/Tile).

Sharding: core c owns head h=c for both batches (16 (b,h) pairs / 8 cores = 2
pairs per core: data-parallel B x tensor-parallel H per the spec hint). Each
core computes its head's q/k/v/a/b projections + short-conv + norms from the
(host-staged) full inputs, runs the gated delta recurrence in chunked form
(C=128, UT transform, truncated-doubling triangular solve, levels=3), then the
cores exchange per-head outputs with an AllToAll so each core finishes
token-parallel (RMS norm + gate + output projection) for its 512-token slice.
A 16KB AllReduce carries the cross-head sum-of-squares for the RMS norm.

Matmuls run in bf16 with fp32 PSUM accumulation (validated absmax/scale ~5e-3
vs the fp32 reference on this problem's data, well under the 2e-2 gate). The
within-chunk decay exponents L are kept to full fp32 precision on the PE by
splitting them into bf16 hi+lo parts and accumulating rank-1 matmuls.

This build works around a walrus codegen limitation in this container
(instructions with >1 sync-wait commands are rejected) by splitting waits
onto same-engine NOPs at Tile commit time.
"""
import sys

sys.path.insert(0, "/opt/trn_rl_repo")
sys.path.insert(0, "/opt/pypackages")

import numpy as np
import ml_dtypes

B, T = 2, 2048
DM, DKV = 1024, 512
H, KH, VH = 8, 96, 192
KQT, VT = H * KH, H * VH
KS = 4
EPS = 1e-6
C = 128                      # chunk length
NCHUNK = T // C              # 16 chunks per pair
NTOK = B * T                 # 4096 tokens
TOKSLC = NTOK // 8           # 512 tokens per core in the final phase
NEG = -30000.0               # additive mask (exp -> 0)

_CACHE = {}


def _build():
    import bass_rust
    from contextlib import ExitStack
    from concourse import bass, mybir
    from concourse.tile import TileContext
    from concourse.vector_clock import ScopedClock

    F32, BF16 = mybir.dt.float32, mybir.dt.bfloat16
    AL = mybir.AluOpType
    AF = mybir.ActivationFunctionType

    # ---- workarounds: walrus rejects >1 sync-wait per instruction ----
    def _drain_patch(self, tick_clock, wait_clock):
        carrier = self.nc.sync.nop(nofuse=True, hint="drain_waits")
        wait_clock.add_sem_waits(
            carrier.ins, ScopedClock({None: tick_clock.global_clock}))
        si = carrier.ins.sync_info
        waits = list(si.on_wait) if si is not None else []
        if len(waits) > 1:
            carrier.ins.sync_info = bass_rust.SyncInfo(
                on_wait=[waits[0]], on_update=[])
            for w in waits[1:]:
                extra = self.nc.sync.nop(nofuse=True, hint="drain_waits")
                extra.ins.sync_info = bass_rust.SyncInfo(
                    on_wait=[w], on_update=[])
        self.nc.sync.drain()
        self.nc.all_engine_barrier()
        popped = self.nc._tile_sem_poison_stack.pop()
        assert popped is self._sem_poison
        self.nc.clear_and_free_semaphores(
            list(self.sems.allocated().values()))
        self.nc.all_engine_barrier()

    TileContext._drain_and_barrier = _drain_patch
    if not getattr(TileContext, "_split_waits_patched", False):
        _orig_commit = TileContext._commit_instruction

        def _commit_split(self, inst, lazy_reg_writes=True):
            si = getattr(inst, "sync_info", None)
            if (si is not None and si.on_wait
                    and inst.engine != mybir.EngineType.Unassigned
                    and (len(si.on_wait) > 1
                         or isinstance(inst, mybir.InstDrain))):
                waits = list(si.on_wait)
                keep = [] if isinstance(inst, mybir.InstDrain) else [waits.pop(0)]
                for w in waits:
                    nop = mybir.InstNoOp(
                        name=self.nc.get_next_instruction_name(),
                        engine=inst.engine, ins=[], outs=[], debug=inst.debug)
                    nop.sync_info = bass_rust.SyncInfo(on_wait=[w], on_update=[])
                    self.nc.register_instruction(nop, overwrite=True)
                    self._add_instruction(nop)
                inst.sync_info = bass_rust.SyncInfo(
                    on_wait=keep, on_update=list(si.on_update))
            return _orig_commit(self, inst, lazy_reg_writes)

        TileContext._commit_instruction = _commit_split
        TileContext._split_waits_patched = True

    nc = bass.Bass()
    P = {}

    def dp(name, shape, dt):
        P[name] = nc.declare_dram_parameter(name, list(shape), dt, isOutput=False)
        return P[name]

    xT = dp("xT", (DM, NTOK), BF16)
    ckvT = dp("ckvT", (DKV, NTOK), BF16)
    wqab = dp("wqab", (DM, KH + 2), BF16)
    wk = dp("wk", (DKV, KH), BF16)
    wv = dp("wv", (DKV, VH), BF16)
    cqd = dp("cqd", (KS, KH, KH), BF16)
    ckd = dp("ckd", (KS, KH, KH), BF16)
    cvd = dp("cvd", (KS, 2, KH, KH), BF16)
    scal = dp("scal", (1, 8), F32)
    gw = dp("gw", (DM, VT), BF16)
    wo = dp("wo", (VT, DM), BF16)
    xsT = dp("xsT", (DM, TOKSLC), BF16)
    png = dp("png", (VT, 1), F32)
    maskS = dp("maskS", (C, C), F32)
    maskI = dp("maskI", (C, C), F32)
    id128f = dp("id128f", (128, 128), F32)
    id128b = dp("id128b", (128, 128), BF16)
    id96b = dp("id96b", (96, 96), BF16)
    id1b = dp("id1b", (1, 1), BF16)
    onesb = dp("onesb", (1, 128), BF16)
    monesb = dp("monesb", (1, 128), BF16)
    one1b = dp("one1b", (1, 1), BF16)
    selq = dp("selq", (B * NCHUNK, 4), F32)
    ones2b = dp("ones2b", (2, 128), BF16)
    mones2b = dp("mones2b", (2, 128), BF16)
    one2b = dp("one2b", (2, 1), BF16)
    id4f = dp("id4f", (4, 4), F32)
    out = nc.declare_dram_parameter("out", [TOKSLC, DM], F32, isOutput=True)

    with TileContext(nc, pool_alloc_mode="queue") as tc:
        ctx = ExitStack()
        cst = ctx.enter_context(tc.tile_pool(name="cst", bufs=1))
        pers = ctx.enter_context(tc.tile_pool(name="pers", bufs=1))
        scr = ctx.enter_context(tc.tile_pool(name="scr", bufs=2))
        ring = ctx.enter_context(tc.tile_pool(name="ring", bufs=3))
        strm = ctx.enter_context(tc.tile_pool(name="strm", bufs=3))
        ps_seq = ctx.enter_context(tc.tile_pool(name="ps_seq", bufs=2, space="PSUM"))
        ps_gate = ctx.enter_context(tc.tile_pool(name="ps_gate", bufs=1, space="PSUM"))
        ps_mm = ctx.enter_context(tc.tile_pool(name="ps_mm", bufs=3, space="PSUM"))
        ps_tiny = ctx.enter_context(tc.tile_pool(name="ps_tiny", bufs=2, space="PSUM"))
        dram = ctx.enter_context(tc.tile_pool(name="dram", bufs=1, space="DRAM"))

        def cload(pool, pname, shape, dt, rearr=None):
            t = pool.tile(list(shape), dt, name=pname + "_s")
            src = P[pname][:]
            if rearr is not None:
                src = src.rearrange(rearr[0], **rearr[1])
            nc.sync.dma_start(out=t[:], in_=src)
            return t

        wqab_s = cload(cst, "wqab", (128, 8, KH + 2), BF16,
                       ("(kc p) f -> p kc f", dict(p=128)))
        wk_s = cload(cst, "wk", (128, 4, KH), BF16,
                     ("(kc p) f -> p kc f", dict(p=128)))
        wv_s = cload(cst, "wv", (128, 4, VH), BF16,
                     ("(kc p) f -> p kc f", dict(p=128)))
        cqd_s = cload(cst, "cqd", (KH, KS, KH), BF16, ("s p f -> p s f", {}))
        ckd_s = cload(cst, "ckd", (KH, KS, KH), BF16, ("s p f -> p s f", {}))
        cvd_s = cload(cst, "cvd", (KH, KS, 2, KH), BF16, ("s h p f -> p s h f", {}))
        scal_s = cload(cst, "scal", (1, 8), F32)
        png_s = cload(cst, "png", (128, 12), F32,
                      ("(ct p) o -> p (ct o)", dict(p=128)))
        maskS_s = cload(cst, "maskS", (C, C), F32)
        maskI_s = cload(cst, "maskI", (C, C), F32)
        id128f_s = cload(cst, "id128f", (128, 128), F32)
        id128b_s = cload(cst, "id128b", (128, 128), BF16)
        id96b_s = cload(cst, "id96b", (96, 96), BF16)
        id1b_s = cload(cst, "id1b", (1, 1), BF16)
        onesb_s = cload(cst, "onesb", (1, 128), BF16)
        monesb_s = cload(cst, "monesb", (1, 128), BF16)
        one1b_s = cload(cst, "one1b", (1, 1), BF16)
        selq_s = cload(cst, "selq", (B * NCHUNK, 4), F32)
        ones2b_s = cload(cst, "ones2b", (2, 128), BF16)
        mones2b_s = cload(cst, "mones2b", (2, 128), BF16)
        one2b_s = cload(cst, "one2b", (2, 1), BF16)
        id4f_s = cload(cst, "id4f", (4, 4), F32)
        ones96b_s = cst.tile([KH, 1], BF16)
        nc.vector.memset(ones96b_s[:], 1.0)
        epsb_s = cst.tile([C, 1], F32)
        nc.vector.memset(epsb_s[:], EPS)
        one1f_s = cst.tile([1, 1], F32)
        nc.vector.memset(one1f_s[:], 1.0)

        # persistent outputs of the scan
        ssqc = pers.tile([C, B * NCHUNK], F32)
        gate = [pers.tile([128, TOKSLC], BF16, name=f"gate{ct}") for ct in range(12)]

        # ---------------- A) projections ----------------
        work_cm = tc.tile_pool(name="work", bufs=1)
        work = work_cm.__enter__()
        rawp_cm = tc.tile_pool(name="rawp", bufs=1)
        rawp = rawp_cm.__enter__()
        qraw = [rawp.tile([KH, 3 + T], BF16, name=f"qraw{p}") for p in range(B)]
        kraw = [rawp.tile([KH, 3 + T], BF16, name=f"kraw{p}") for p in range(B)]
        v0raw = [rawp.tile([KH, 3 + T], BF16, name=f"v0raw{p}") for p in range(B)]
        v1raw = [rawp.tile([KH, 3 + T], BF16, name=f"v1raw{p}") for p in range(B)]
        abd = [dram.tile([2, T], F32, name=f"abd{p}") for p in range(B)]
        for p in range(B):
            for t_ in (qraw[p], kraw[p], v0raw[p], v1raw[p]):
                nc.vector.memset(t_[:, 0:3], 0.0)

        for tt in range(8):
            p, lt = tt // 4, tt % 4
            ps_q = ps_seq.tile([KH + 2, 512], F32, tag="seq")
            for kc in range(8):
                xt = strm.tile([128, 512], BF16, tag="xt")
                nc.gpsimd.dma_start(
                    out=xt[:], in_=xT[kc * 128:(kc + 1) * 128,
                                      tt * 512:(tt + 1) * 512])
                nc.tensor.matmul(ps_q[:], lhsT=wqab_s[:, kc, :],
                                 rhs=xt[:], start=(kc == 0), stop=(kc == 7))
            nc.vector.tensor_copy(out=qraw[p][:, 3 + lt * 512: 3 + (lt + 1) * 512],
                           in_=ps_q[0:KH, :])
            abev = scr.tile([2, 512], F32, tag="abev", bufs=1)
            nc.vector.tensor_copy(out=abev[:], in_=ps_q[KH:KH + 2, :])
            nc.sync.dma_start(out=abd[p][:, lt * 512:(lt + 1) * 512], in_=abev[:])

            ps_k = ps_seq.tile([KH, 512], F32, tag="seq")
            ps_v0 = ps_mm.tile([KH, 512], F32, tag="mm")
            ps_v1 = ps_mm.tile([KH, 512], F32, tag="mm")
            for kc in range(4):
                ct = strm.tile([128, 512], BF16, tag="ct")
                nc.gpsimd.dma_start(
                    out=ct[:], in_=ckvT[kc * 128:(kc + 1) * 128,
                                        tt * 512:(tt + 1) * 512])
                nc.tensor.matmul(ps_k[:], lhsT=wk_s[:, kc, :],
                                 rhs=ct[:], start=(kc == 0), stop=(kc == 3))
                nc.tensor.matmul(ps_v0[:], lhsT=wv_s[:, kc, 0:KH],
                                 rhs=ct[:], start=(kc == 0), stop=(kc == 3))
                nc.tensor.matmul(ps_v1[:], lhsT=wv_s[:, kc, KH:VH],
                                 rhs=ct[:], start=(kc == 0), stop=(kc == 3))
            nc.vector.tensor_copy(out=kraw[p][:, 3 + lt * 512: 3 + (lt + 1) * 512],
                           in_=ps_k[:])
            nc.vector.tensor_copy(out=v0raw[p][:, 3 + lt * 512: 3 + (lt + 1) * 512],
                           in_=ps_v0[:])
            nc.vector.tensor_copy(out=v1raw[p][:, 3 + lt * 512: 3 + (lt + 1) * 512],
                           in_=ps_v1[:])

        # ---------------- B) conv + SiLU (bf16 outs) ----------------
        qn = [work.tile([KH, T], BF16, name=f"qn{p}") for p in range(B)]
        kn = [work.tile([KH, T], BF16, name=f"kn{p}") for p in range(B)]
        vc0 = [work.tile([KH, T], BF16, name=f"vc0{p}") for p in range(B)]
        vc1 = [work.tile([KH, T], BF16, name=f"vc1{p}") for p in range(B)]
        conv_jobs = []
        for p in range(B):
            conv_jobs += [(qraw[p], cqd_s, qn[p], None),
                          (kraw[p], ckd_s, kn[p], None),
                          (v0raw[p], cvd_s, vc0[p], 0),
                          (v1raw[p], cvd_s, vc1[p], 1)]
        for raw, dg, dst, vh in conv_jobs:
            for lt in range(4):
                ps_c = ps_mm.tile([KH, 512], F32, tag="mm")
                for s in range(KS):
                    lhs = dg[:, s, vh, :] if vh is not None else dg[:, s, :]
                    nc.tensor.matmul(ps_c[:], lhsT=lhs,
                                     rhs=raw[:, lt * 512 + s: lt * 512 + s + 512],
                                     start=(s == 0), stop=(s == KS - 1))
                nc.scalar.activation(out=dst[:, lt * 512:(lt + 1) * 512],
                                     in_=ps_c[:], func=AF.Silu)
        rawp_cm.__exit__(None, None, None)

        # gate weights reuse the raw tiles' space (gate GEMM overlaps the scan)
        mid_cm = tc.tile_pool(name="mid", bufs=1)
        mid = mid_cm.__enter__()
        gw_s = [mid.tile([128, VT], BF16, name=f"gwt{kc}") for kc in range(8)]
        xs_s = [mid.tile([128, TOKSLC], BF16, name=f"xst{kc}") for kc in range(8)]
        for kcb in range(8):
            nc.gpsimd.dma_start(out=gw_s[kcb][:],
                                in_=gw[kcb * 128:(kcb + 1) * 128, :])
            nc.gpsimd.dma_start(out=xs_s[kcb][:],
                                in_=xsT[kcb * 128:(kcb + 1) * 128, :])

        # ---------------- C) L2 norm of q,k (in place) ----------------
        for p in range(B):
            for src, qscale in ((qn[p], KH ** -0.5), (kn[p], None)):
                for lt in range(4):
                    sl = slice(lt * 512, (lt + 1) * 512)
                    sq = scr.tile([KH, 512], BF16, tag="sq")
                    nc.vector.tensor_tensor(out=sq[:], in0=src[:, sl],
                                            in1=src[:, sl], op=AL.mult)
                    ps_n = ps_tiny.tile([1, 512], F32, tag="tiny")
                    nc.tensor.matmul(ps_n[:], lhsT=ones96b_s[:], rhs=sq[:],
                                     start=True, stop=True)
                    # factor = qscale/sqrt(ssq) = rsqrt(ssq/qscale^2): one ACT
                    # Rsqrt (eps negligible: ||q|| >> 1e-6 for silu-conv outs).
                    nrb = scr.tile([1, 512], BF16, tag="recb")
                    iscale = (1.0 / float(qscale) ** 2 if qscale is not None
                              else 1.0)
                    eng = nc.scalar
                    eng.add_instruction(mybir.InstActivation(
                        name=nc.get_next_instruction_name(),
                        func=AF.Rsqrt,
                        ins=[eng.lower_ap(ps_n[:]),
                             mybir.ImmediateValue(dtype=F32, value=0.0),
                             mybir.ImmediateValue(dtype=F32, value=iscale),
                             mybir.ImmediateValue(dtype=F32, value=0.0)],
                        outs=[eng.lower_ap(nrb[:])]))
                    ps_f = ps_mm.tile([KH, 512], F32, tag="mm")
                    nc.tensor.matmul(ps_f[:], lhsT=onesb_s[0:1, 0:KH],
                                     rhs=nrb[:], start=True, stop=True)
                    nc.vector.tensor_tensor(out=src[:, sl], in0=src[:, sl],
                                            in1=ps_f[:], op=AL.mult)

        # ---------------- D) decay rows + cumsum (bf16 hi/lo rows) -----------
        Lhl = [work.tile([2, T], BF16, name=f"Lhl{p}") for p in range(B)]
        Lh = [work.tile([1, T], BF16, name=f"Lh{p}") for p in range(B)]
        Ll = [work.tile([1, T], BF16, name=f"Ll{p}") for p in range(B)]
        brow = [work.tile([1, T], BF16, name=f"brow{p}") for p in range(B)]
        lnbrow = [work.tile([1, T], BF16, name=f"lnbrow{p}") for p in range(B)]
        for p in range(B):
            arow = scr.tile([1, T], F32, tag="rowC", bufs=1)
            nc.sync.dma_start(out=arow[:], in_=abd[p][0:1, :])
            rsc0 = scr.tile([1, T], F32, tag="rowD", bufs=1)
            nc.scalar.activation(out=rsc0[:], in_=arow[:], func=AF.Exp,
                                 bias=scal_s[0:1, 0:1], scale=1.0)
            rsc = scr.tile([1, T], F32, tag="rowC", bufs=1)
            nc.scalar.activation(out=rsc[:], in_=rsc0[:], func=AF.Ln,
                                 bias=one1f_s[:], scale=1.0)
            brawrow = scr.tile([1, T], F32, tag="rowC", bufs=1)
            nc.sync.dma_start(out=brawrow[:], in_=abd[p][1:2, :])
            grow = scr.tile([1, T], F32, tag="rowB", bufs=1)
            nc.vector.tensor_scalar(out=grow[:], in0=rsc[:],
                                    scalar1=scal_s[0:1, 1:2], scalar2=None,
                                    op0=AL.mult)
            nc.scalar.activation(out=brow[p][:], in_=brawrow[:],
                                 func=AF.Sigmoid, bias=scal_s[0:1, 2:3], scale=1.0)
            lnb_e = scr.tile([1, T], F32, tag="rowD", bufs=1)
            nc.scalar.activation(out=lnb_e[:], in_=brawrow[:], func=AF.Exp,
                                 bias=scal_s[0:1, 3:4], scale=-1.0)
            lnb_t = scr.tile([1, T], F32, tag="rowC", bufs=1)
            nc.scalar.activation(out=lnb_t[:], in_=lnb_e[:], func=AF.Ln,
                                 bias=one1f_s[:], scale=1.0)
            nc.vector.tensor_scalar_mul(out=lnbrow[p][:], in0=lnb_t[:], scalar1=-1.0)
            gb = dram.tile([NCHUNK, C], F32, name=f"gb{p}")
            nc.sync.dma_start(
                out=gb[:].rearrange("p f -> (p f)").unsqueeze(0), in_=grow[:])
            g16 = scr.tile([NCHUNK, C], F32, tag="g16")
            nc.sync.dma_start(out=g16[:], in_=gb[:])
            L16t = scr.tile([NCHUNK, C], F32, tag="L16t")
            nc.vector.tensor_tensor_scan(out=L16t[:], data0=g16[:], data1=g16[:],
                                         initial=0.0, op0=AL.add, op1=AL.bypass)
            gb2 = dram.tile([NCHUNK, C], F32, name=f"gb2{p}")
            nc.sync.dma_start(out=gb2[:], in_=L16t[:])
            Lrow = scr.tile([1, T], F32, tag="rowC", bufs=1)
            nc.sync.dma_start(
                out=Lrow[:], in_=gb2[:].rearrange("p f -> (p f)").unsqueeze(0))
            nc.vector.tensor_copy(out=Lh[p][:], in_=Lrow[:])
            nc.vector.tensor_tensor(out=Ll[p][:], in0=Lrow[:], in1=Lh[p][:],
                                    op=AL.subtract)
            lbd = dram.tile([2, T], BF16, name=f"lbd{p}")
            nc.sync.dma_start(out=lbd[0:1, :], in_=Lh[p][:])
            nc.sync.dma_start(out=lbd[1:2, :], in_=Ll[p][:])
            nc.sync.dma_start(out=Lhl[p][:], in_=lbd[:])

        # -------- E/F/G) per-chunk prep + sequential sweep + output ----------
        a2ain = [dram.tile([8, VH, TOKSLC // 2], BF16, name=f"a2ain{h_}")
                 for h_ in range(2)]
        a2aout = [dram.tile([8, VH, TOKSLC // 2], BF16, name=f"a2aout{h_}")
                  for h_ in range(2)]
        Scur = []
        for p in range(B):
            s0 = ring.tile([KH, VH], BF16, tag=f"Sh{p}", bufs=4, name=f"S0_{p}")
            nc.vector.memset(s0[:], 0.0)
            Scur.append(s0)

        for i in range(NCHUNK):
            for p in range(B):
                j = i * B + p
                ck = slice(i * C, (i + 1) * C)
                lstc = slice((i + 1) * C - 1, (i + 1) * C)
                Lrh, Lrl = Lh[p][0:1, ck], Ll[p][0:1, ck]

                # E[t,s] = L_t - L_s via 4 rank-1 bf16 matmuls (hi/lo
                # exact); E, Gkk, Gqk packed into one psum bank.
                ps_egg = ps_mm.tile([C, 3, C], F32, tag="mm")
                ps_e = ps_egg[:, 0]
                nc.tensor.matmul(ps_e, lhsT=Lhl[p][:, ck], rhs=ones2b_s[:],
                                 start=True, stop=False)
                nc.tensor.matmul(ps_e, lhsT=mones2b_s[:], rhs=Lhl[p][:, ck],
                                 start=False, stop=True)
                Es = scr.tile([C, C], F32, tag="Es", bufs=3)
                nc.vector.tensor_tensor(out=Es[:], in0=ps_e, in1=maskS_s[:],
                                        op=AL.add)
                Ei = scr.tile([C, C], F32, tag="Ei", bufs=3)
                nc.vector.tensor_tensor(out=Ei[:], in0=ps_e, in1=maskI_s[:],
                                        op=AL.add)

                # lnb / beta columns
                ps_lb = ps_tiny.tile([C, 4], BF16, tag="tiny")
                nc.tensor.transpose(ps_lb[:, 0:1], lnbrow[p][0:1, ck], id1b_s[:])
                nc.tensor.transpose(ps_lb[:, 2:3], brow[p][0:1, ck], id1b_s[:])
                lnbc = scr.tile([C, 1], F32, tag="lnbc")
                nc.scalar.copy(out=lnbc[:], in_=ps_lb[:, 0:1])
                bc = scr.tile([C, 1], F32, tag="bc")
                nc.scalar.copy(out=bc[:], in_=ps_lb[:, 2:3])

                Mexp = scr.tile([C, C], F32, tag="Mexp", bufs=3)
                nc.scalar.activation(out=Mexp[:], in_=Es[:], func=AF.Exp,
                                     bias=lnbc[:], scale=1.0)
                Dincl = scr.tile([C, C], F32, tag="Dincl", bufs=3)
                nc.scalar.activation(out=Dincl[:], in_=Ei[:], func=AF.Exp)

                # N (strict-lower, includes beta) and its transpose
                ps_gk = ps_egg[:, 1]
                nc.tensor.matmul(ps_gk, lhsT=kn[p][:, ck], rhs=kn[p][:, ck],
                                 start=True, stop=True)
                Nbf = scr.tile([C, C], BF16, tag="Nbf", bufs=3)
                nc.vector.scalar_tensor_tensor(out=Nbf[:], in0=ps_gk,
                                               scalar=-1.0, in1=Mexp[:],
                                               op0=AL.mult, op1=AL.mult)
                # P = (q_t . k_s) * Dincl
                ps_gq = ps_egg[:, 2]
                nc.tensor.matmul(ps_gq, lhsT=qn[p][:, ck], rhs=kn[p][:, ck],
                                 start=True, stop=True)
                Pbf = scr.tile([C, C], BF16, tag="Pbf", bufs=3)
                nc.vector.tensor_tensor(out=Pbf[:], in0=ps_gq, in1=Dincl[:],
                                        op=AL.mult)
                # both transposes (N, P) share one psum bank
                ps_nt = ps_mm.tile([C, 2, C], BF16, tag="mm")
                nc.tensor.transpose(ps_nt[:, 0], Nbf[:], id128b_s[:])
                nc.tensor.transpose(ps_nt[:, 1], Pbf[:], id128b_s[:])
                ev = nc.scalar.copy if (j % 2 == 0) else \
                    (lambda out, in_: nc.vector.tensor_copy(out=out, in_=in_))
                NTbf = scr.tile([C, C], BF16, tag="NTbf", bufs=3)
                ev(out=NTbf[:], in_=ps_nt[:, 0])
                P0b = scr.tile([C, C], BF16, tag="P0b", bufs=3)
                nc.vector.tensor_tensor(out=P0b[:], in0=ps_nt[:, 0],
                                        in1=id128f_s[:], op=AL.add)
                PTb = ring.tile([C, C], BF16, tag="PTb", name=f"PTb{j}")
                nc.scalar.copy(out=PTb[:], in_=ps_nt[:, 1])

                # doubling powers (levels=3)
                ps_sq = ps_mm.tile([C, 2, C], F32, tag="mm")
                nc.tensor.matmul(ps_sq[:, 0], lhsT=NTbf[:], rhs=Nbf[:],
                                 start=True, stop=True)
                nc.tensor.matmul(ps_sq[:, 1], lhsT=Nbf[:], rhs=NTbf[:],
                                 start=True, stop=True)
                N2r = scr.tile([C, C], BF16, tag="N2r", bufs=3)
                ev(out=N2r[:], in_=ps_sq[:, 0])
                N2i = scr.tile([C, C], BF16, tag="N2i", bufs=3)
                nc.vector.tensor_tensor(out=N2i[:], in0=ps_sq[:, 0],
                                        in1=id128f_s[:], op=AL.add)
                N2Tr = scr.tile([C, C], BF16, tag="N2Tr", bufs=3)
                ev(out=N2Tr[:], in_=ps_sq[:, 1])
                ps_sq3 = ps_mm.tile([C, 2, C], F32, tag="mm")
                nc.tensor.matmul(ps_sq3[:, 0], lhsT=N2Tr[:], rhs=N2r[:],
                                 start=True, stop=True)
                nc.tensor.matmul(ps_sq3[:, 1], lhsT=N2r[:], rhs=N2Tr[:],
                                 start=True, stop=True)
                N4r = scr.tile([C, C], BF16, tag="N4r", bufs=3)
                ev(out=N4r[:], in_=ps_sq3[:, 0])
                N4i = scr.tile([C, C], BF16, tag="N4i", bufs=3)
                nc.vector.tensor_tensor(out=N4i[:], in0=ps_sq3[:, 0],
                                        in1=id128f_s[:], op=AL.add)
                N4Tr = scr.tile([C, C], BF16, tag="N4Tr", bufs=3)
                ev(out=N4Tr[:], in_=ps_sq3[:, 1])
                ps_sq5 = ps_mm.tile([C, C], F32, tag="mm")
                nc.tensor.matmul(ps_sq5[:], lhsT=N4Tr[:], rhs=N4r[:],
                                 start=True, stop=True)
                N8i = scr.tile([C, C], BF16, tag="N8i", bufs=3)
                nc.vector.tensor_tensor(out=N8i[:], in0=ps_sq5[:], in1=id128f_s[:],
                                        op=AL.add)

                # chain: T^T = (I+N8T)(I+N4T)(I+N2T)(I+NT)
                ps_c1 = ps_mm.tile([C, C], F32, tag="mm")
                nc.tensor.matmul(ps_c1[:], lhsT=N2i[:], rhs=P0b[:],
                                 start=True, stop=True)
                C1 = scr.tile([C, C], BF16, tag="C1", bufs=3)
                ev(out=C1[:], in_=ps_c1[:])
                ps_c2 = ps_mm.tile([C, C], F32, tag="mm")
                nc.tensor.matmul(ps_c2[:], lhsT=N4i[:], rhs=C1[:],
                                 start=True, stop=True)
                C2 = scr.tile([C, C], BF16, tag="C2", bufs=3)
                ev(out=C2[:], in_=ps_c2[:])
                ps_c3 = ps_mm.tile([C, C], F32, tag="mm")
                nc.tensor.matmul(ps_c3[:], lhsT=N8i[:], rhs=C2[:],
                                 start=True, stop=True)
                TTm = ring.tile([C, C], BF16, tag="TTm", name=f"TTm{j}")
                nc.scalar.copy(out=TTm[:], in_=ps_c3[:])

                # V halves + K token-layout transposes share one psum bank
                bV = ring.tile([C, VH], BF16, tag="bV", name=f"bV{j}")
                ps_vt = ps_mm.tile([C, 3, KH], BF16, tag="mm")
                nc.tensor.transpose(ps_vt[:, 0], vc0[p][:, ck], id96b_s[:])
                nc.tensor.transpose(ps_vt[:, 1], vc1[p][:, ck], id96b_s[:])
                nc.tensor.transpose(ps_vt[:, 2], kn[p][:, ck], id96b_s[:])
                for hh in range(2):
                    nc.vector.tensor_scalar(out=bV[:, hh * KH:(hh + 1) * KH],
                                            in0=ps_vt[:, hh], scalar1=bc[:],
                                            scalar2=None, op0=AL.mult)
                ps_kt = ps_vt[:, 2]
                ps_ll3 = ps_tiny.tile([C, 3], F32, tag="tiny")
                nc.tensor.matmul(ps_ll3[:, 0:1], lhsT=ones2b_s[:],
                                 rhs=Lhl[p][:, lstc], start=True, stop=True)
                Llc = scr.tile([C, 1], F32, tag="Llc")
                nc.scalar.copy(out=Llc[:], in_=ps_ll3[:, 0:1])
                nc.tensor.matmul(ps_ll3[:, 1:2], lhsT=Lhl[p][:, ck],
                                 rhs=one2b_s[:], start=True, stop=True)
                eLl = scr.tile([C, 1], F32, tag="eLl")
                nc.scalar.activation(out=eLl[:], in_=ps_ll3[:, 1:2], func=AF.Exp,
                                     bias=Llc[:], scale=-1.0)
                Ktok = ring.tile([C, KH], BF16, tag="Ktok", name=f"Ktok{j}")
                nc.vector.tensor_scalar(out=Ktok[:], in0=ps_kt, scalar1=eLl[:],
                                        scalar2=None, op0=AL.mult)

                # A_t column, beta*A column, chunk-total decay
                Acol = ring.tile([C, 1], F32, tag="Acol", name=f"Acol{j}")
                nc.scalar.activation(out=Acol[:], in_=ps_ll3[:, 1:2], func=AF.Exp)
                bAcol = ring.tile([C, 1], F32, tag="bAcol", name=f"bAcol{j}")
                nc.vector.tensor_tensor(out=bAcol[:], in0=Acol[:], in1=bc[:],
                                        op=AL.mult)
                nc.tensor.matmul(ps_ll3[0:KH, 2:3], lhsT=ones2b_s[:, 0:KH],
                                 rhs=Lhl[p][:, lstc], start=True, stop=True)
                aC96 = ring.tile([KH, 1], F32, tag="aC96", name=f"aC96{j}")
                nc.scalar.activation(out=aC96[:], in_=ps_ll3[0:KH, 2:3],
                                     func=AF.Exp)

                # ---- sequential sweep step ----
                ps_y = ps_seq.tile([C, VH], F32, tag="seq")
                nc.tensor.matmul(ps_y[:], lhsT=kn[p][:, ck], rhs=Scur[p][:],
                                 start=True, stop=True)
                R2 = scr.tile([C, VH], BF16, tag="R2")
                nc.vector.scalar_tensor_tensor(out=R2[:], in0=ps_y[:],
                                               scalar=bAcol[:], in1=bV[:],
                                               op0=AL.mult, op1=AL.subtract)
                ps_u = ps_seq.tile([C, VH], F32, tag="seq")
                nc.tensor.matmul(ps_u[:], lhsT=TTm[:], rhs=R2[:],
                                 start=True, stop=True)
                U = ring.tile([C, VH], BF16, tag="U", name=f"U{j}")
                nc.scalar.activation(out=U[:], in_=ps_u[:], func=AF.Copy,
                                     scale=-1.0)
                ps_s = ps_seq.tile([KH, VH], F32, tag="seq")
                nc.tensor.matmul(ps_s[:], lhsT=Ktok[:], rhs=U[:],
                                 start=True, stop=True)
                Snew = ring.tile([KH, VH], BF16, tag=f"Sh{p}", bufs=4,
                                 name=f"S{p}_{i + 1}")
                nc.vector.scalar_tensor_tensor(out=Snew[:], in0=Scur[p][:],
                                               scalar=aC96[:], in1=ps_s[:],
                                               op0=AL.mult, op1=AL.add)

                # ---- output epilogue ----
                ps_pu = ps_seq.tile([C, VH], F32, tag="seq")
                nc.tensor.matmul(ps_pu[:], lhsT=PTb[:], rhs=U[:],
                                 start=True, stop=True)
                ps_z = ps_seq.tile([C, VH], F32, tag="seq")
                nc.tensor.matmul(ps_z[:], lhsT=qn[p][:, ck], rhs=Scur[p][:],
                                 start=True, stop=True)
                tmpZ = scr.tile([C, VH], F32, tag="tmpZ")
                nc.vector.tensor_scalar(out=tmpZ[:], in0=ps_z[:], scalar1=Acol[:],
                                        scalar2=None, op0=AL.mult)
                Osb = scr.tile([C, VH], BF16, tag="Osb", bufs=3)
                nc.vector.tensor_tensor(out=Osb[:], in0=ps_pu[:], in1=tmpZ[:],
                                        op=AL.add)
                sqo = scr.tile([C, VH], BF16, tag="sqo")
                nc.vector.scalar_tensor_tensor(out=sqo[:], in0=Osb[:],
                                               scalar=1.0, in1=Osb[:],
                                               op0=AL.mult, op1=AL.mult,
                                               accum_out=ssqc[:, j:j + 1])
                # transpose + ship (unnormalized) o to the A2A send buffer;
                # the RMS factor is applied per-token at the final GEMM evict.
                tok0 = p * T + i * C
                d, off = tok0 // TOKSLC, tok0 % TOKSLC
                half, off2 = (0, off) if off < TOKSLC // 2 else (1, off - TOKSLC // 2)
                ps_ot = ps_seq.tile([KH, 2, C], BF16, tag="seq")
                nc.tensor.transpose(ps_ot[:, 0], Osb[:, 0:KH], id128b_s[:])
                nc.tensor.transpose(ps_ot[:, 1], Osb[:, KH:VH], id128b_s[:])
                for hh in range(2):
                    otb = scr.tile([KH, C], BF16, tag="otb")
                    ev(out=otb[:], in_=ps_ot[:, hh])
                    nc.sync.dma_start(
                        out=a2ain[half][d, hh * KH:(hh + 1) * KH, off2:off2 + C],
                        in_=otb[:])
                Scur[p] = Snew
                if i == NCHUNK - 3 and p == B - 1:
                    # all chunks of the first token-half are now emitted
                    nc.gpsimd.collective_compute(
                        "AllToAll", AL.bypass, replica_groups=[list(range(8))],
                        ins=[a2ain[0].opt()], outs=[a2aout[0].opt()])

        # ---------------- K) gate GEMM (overlaps scan via dataflow) ----------
        for ctb in range(12):
            ps_g = ps_gate.tile([128, TOKSLC], F32, tag="gate")
            for kcb in range(8):
                nc.tensor.matmul(ps_g[:],
                                 lhsT=gw_s[kcb][:, ctb * 128:(ctb + 1) * 128],
                                 rhs=xs_s[kcb][:], start=(kcb == 0), stop=(kcb == 7))
            nc.scalar.activation(out=gate[ctb][:], in_=ps_g[:], func=AF.Silu)

        # ---------------- H) ssq AllReduce + rsqrt ----------------
        arin = dram.tile([C, B * NCHUNK], F32, name="arin")
        arout = dram.tile([C, B * NCHUNK], F32, name="arout")
        nc.sync.dma_start(out=arin[:], in_=ssqc[:])
        nc.gpsimd.collective_compute(
            "AllReduce", AL.add, replica_groups=[list(range(8))],
            ins=[arin.opt()], outs=[arout.opt()])
        rq = pers.tile([C, B * NCHUNK], F32)
        nc.sync.dma_start(out=rq[:], in_=arout[:])
        rb = pers.tile([C, B * NCHUNK], F32)
        nc.scalar.activation(out=rb[:], in_=rq[:], func=AF.Sqrt,
                             bias=epsb_s[:], scale=1.0 / VT)
        rs = pers.tile([C, B * NCHUNK], F32)
        nc.vector.reciprocal(out=rs[:], in_=rb[:])
        # gather this core's 4 per-token-block rs columns via one-hot matmul
        ps_rt = ps_tiny.tile([B * NCHUNK, C], F32, tag="tiny")
        nc.tensor.transpose(ps_rt[:], rs[:], id128f_s[:])
        rsT = pers.tile([B * NCHUNK, C], F32)
        nc.scalar.copy(out=rsT[:], in_=ps_rt[:])
        ps_r4 = ps_tiny.tile([4, C], F32, tag="tiny")
        nc.tensor.matmul(ps_r4[:], lhsT=selq_s[:], rhs=rsT[:],
                         start=True, stop=True)
        rs4T = pers.tile([4, C], F32)
        nc.scalar.copy(out=rs4T[:], in_=ps_r4[:])
        ps_rq = ps_tiny.tile([C, 4], F32, tag="tiny")
        nc.tensor.transpose(ps_rq[:], rs4T[:], id4f_s[:])
        rsq_sb = pers.tile([C, 4], F32)
        nc.scalar.copy(out=rsq_sb[:], in_=ps_rq[:])
        rsq = [rsq_sb[:, to:to + 1] for to in range(4)]

        # second-half A2A (first half fires inside the chunk loop)
        nc.gpsimd.collective_compute(
            "AllToAll", AL.bypass, replica_groups=[list(range(8))],
            ins=[a2ain[1].opt()], outs=[a2aout[1].opt()])

        mid_cm.__exit__(None, None, None)
        work_cm.__exit__(None, None, None)

        # ---------------- L) OG product + final GEMM ----------------
        late_cm = tc.tile_pool(name="late", bufs=1)
        late = late_cm.__enter__()
        wo_s = [late.tile([128, DM], BF16, name=f"wot{ct}") for ct in range(12)]
        for ct in range(12):
            nc.gpsimd.dma_start(out=wo_s[ct][:],
                                in_=wo[ct * 128:(ct + 1) * 128, :])
        og = [late.tile([128, TOKSLC], BF16, name=f"og{ct}") for ct in range(12)]
        HT = TOKSLC // 2
        flats = [a2aout[h_][:].rearrange("h c t -> (h c) t") for h_ in range(2)]
        for half in range(2):
            for ct in range(12):
                ogin = late.tile([128, HT], BF16, tag="ogin", bufs=3,
                                 name=f"ogin{half}_{ct}")
                nc.gpsimd.dma_start(out=ogin[:],
                                    in_=flats[half][ct * 128:(ct + 1) * 128, :])
                nc.vector.scalar_tensor_tensor(
                    out=og[ct][:, half * HT:(half + 1) * HT], in0=ogin[:],
                    scalar=png_s[:, ct:ct + 1],
                    in1=gate[ct][:, half * HT:(half + 1) * HT],
                    op0=AL.mult, op1=AL.mult)
        for to in range(4):
            # rs column for this 128-token block of this core's slice
            # (core id is data-independent: token block -> (p, i) -> ssqc col)
            for fo in range(2):
                ps_o = ps_gate.tile([128, 512], F32, tag="gate")
                for ct in range(12):
                    nc.tensor.matmul(ps_o[:],
                                     lhsT=og[ct][:, to * 128:(to + 1) * 128],
                                     rhs=wo_s[ct][:, fo * 512:(fo + 1) * 512],
                                     start=(ct == 0), stop=(ct == 11))
                osb = late.tile([128, 512], F32, tag="osb", bufs=2,
                                name=f"osb{to}_{fo}")
                nc.vector.tensor_scalar(out=osb[:], in0=ps_o[:],
                                        scalar1=rsq[to][:], scalar2=None,
                                        op0=AL.mult)
                nc.sync.dma_start(
                    out=out[to * 128:(to + 1) * 128, fo * 512:(fo + 1) * 512],
                    in_=osb[:])
        late_cm.__exit__(None, None, None)
        ctx.close()

    return nc


def kernel(x, c_kv, w_q, w_k, w_v, conv_q_w, conv_q_b, conv_k_w, conv_k_b,
           conv_v_w, conv_v_b, a_proj_w, a_proj_b, A_log, dt_bias,
           b_proj_w, b_proj_b, g_proj_w, post_norm_w, w_o):
    from concourse.bass_utils import run_bass_kernel_spmd

    bf = ml_dtypes.bfloat16
    x = np.asarray(x, np.float32)
    c_kv = np.asarray(c_kv, np.float32)
    xT = np.ascontiguousarray(x.reshape(NTOK, DM).T).astype(bf)
    ckvT = np.ascontiguousarray(c_kv.reshape(NTOK, DKV).T).astype(bf)
    gw = np.asarray(g_proj_w, np.float32).astype(bf)
    wo_ = np.asarray(w_o, np.float32).astype(bf)
    png = np.asarray(post_norm_w, np.float32).reshape(VT, 1)

    maskS = np.where(np.arange(C)[None, :] < np.arange(C)[:, None], 0.0, NEG)
    maskI = np.where(np.arange(C)[None, :] <= np.arange(C)[:, None], 0.0, NEG)
    consts = dict(
        maskS=maskS.astype(np.float32), maskI=maskI.astype(np.float32),
        id128f=np.eye(128, dtype=np.float32),
        id128b=np.eye(128, dtype=np.float32).astype(bf),
        id96b=np.eye(96, dtype=np.float32).astype(bf),
        id1b=np.ones((1, 1), np.float32).astype(bf),
        onesb=np.ones((1, 128), np.float32).astype(bf),
        monesb=(-np.ones((1, 128), np.float32)).astype(bf),
        one1b=np.ones((1, 1), np.float32).astype(bf),
        id4f=np.eye(4, dtype=np.float32),
        ones2b=np.ones((2, 128), np.float32).astype(bf),
        mones2b=(-np.ones((2, 128), np.float32)).astype(bf),
        one2b=np.ones((2, 1), np.float32).astype(bf),
    )

    in_maps = []
    for c in range(8):
        h = c
        qs = slice(h * KH, (h + 1) * KH)
        vs = slice(h * VH, (h + 1) * VH)
        wqab_ = np.concatenate([
            np.asarray(w_q, np.float32)[:, qs],
            np.asarray(a_proj_w, np.float32)[:, h:h + 1],
            np.asarray(b_proj_w, np.float32)[:, h:h + 1]], axis=1).astype(bf)
        cq = np.asarray(conv_q_w, np.float32)[qs, 0, :]
        ck = np.asarray(conv_k_w, np.float32)[qs, 0, :]
        cv = np.asarray(conv_v_w, np.float32)[vs, 0, :]
        cqd_ = np.stack([np.diag(cq[:, s]) for s in range(KS)]).astype(bf)
        ckd_ = np.stack([np.diag(ck[:, s]) for s in range(KS)]).astype(bf)
        cvd_ = np.stack([np.stack([np.diag(cv[hh * KH:(hh + 1) * KH, s])
                                   for hh in range(2)])
                         for s in range(KS)]).astype(bf)
        scal_ = np.zeros((1, 8), np.float32)
        scal_[0, 0] = float(np.asarray(dt_bias)[h] + np.asarray(a_proj_b)[h])
        scal_[0, 1] = -float(np.exp(np.asarray(A_log)[h]))
        scal_[0, 2] = float(np.asarray(b_proj_b)[h])
        scal_[0, 3] = -float(np.asarray(b_proj_b)[h])
        selq = np.zeros((B * NCHUNK, 4), np.float32)
        for to in range(4):
            tok0 = c * TOKSLC + to * 128
            p_, i_ = tok0 // T, (tok0 % T) // C
            selq[i_ * B + p_, to] = 1.0
        m = dict(
            selq=selq,
            xT=xT, ckvT=ckvT, wqab=wqab_,
            wk=np.asarray(w_k, np.float32)[:, qs].astype(bf),
            wv=np.asarray(w_v, np.float32)[:, vs].astype(bf),
            cqd=cqd_, ckd=ckd_, cvd=cvd_, scal=scal_, gw=gw, wo=wo_,
            xsT=np.ascontiguousarray(xT[:, c * TOKSLC:(c + 1) * TOKSLC]),
            png=png, **consts)
        in_maps.append(m)

    if "nc" not in _CACHE:
        _CACHE["nc"] = _build()
    res = run_bass_kernel_spmd(_CACHE["nc"], in_maps, core_ids=list(range(8)))
    _CACHE["last"] = res
    parts = [np.asarray(res.results[c]["out"], np.float32) for c in range(8)]
    return np.concatenate(parts, axis=0).reshape(B, T, DM)
